# revision 10
# baseline (speedup 1.0000x reference)
"""Trainium2 Bass kernel for a fused multi-head attention block.

Reference computation (B=4, T=2048, D=1152, H=8, HD=144, full rotary):
    q,k,v = x@Wq.T, x@Wk.T, x@Wv.T   (per head)
    q,k   = rope(q, k, cos, sin)
    o     = softmax(q k^T / sqrt(HD)) v
    out   = o @ Wo.T

Sharding (8 cores): core c = (batch b = c//2, head-group hg = c%2).
Each core computes 4 heads of one batch and a partial output
out_part = o_local @ Wo[:, hg_cols].T ; host sums the two partials per batch.

Per-core layout decisions:
  * Host passes x transposed (xT [D, T]) and weights pre-transposed so that
    every matmul contraction sits on the partition axis.
  * q/k head dims are padded 144 -> 160 and reordered on the host into
    [h0:0-127 | h1:0-127 | h2:0-127 | h3:0-127 | b-block 4x(16 real + 16 zero)]
    so that per-head tiles stay 128/32-partition aligned on chip.
  * Scores are computed transposed (S^T [keys, q]) so the PV matmul needs no
    transpose, and the softmax denominator comes free by appending a ones
    column to v (o_psum[:, 144] = sum(exp(S))).
  * exp() has no max-subtraction: scores*scale have std ~0.7, |S|<6, safely
    inside fp32/bf16 exp range.
  * dtypes: projections/final in f32r (fp32 bits, fast PE path), attention
    matmuls in bf16, all accumulation fp32 in PSUM.
"""

import numpy as np

B, T, D, H = 4, 2048, 1152, 8
HL = 4              # heads per core
HD = 144            # head dim
EP = 640            # padded q/k projection width: 4*128 + 128 (4x(16+16pad))
DV = HL * HD        # 576, v/o width
NT = T // 128       # 16 t-tiles
KC = D // 128       # 9 contraction chunks
SCALE = float(HD) ** -0.5
NCORES = 8

_NC_CACHE = {}
GSZ = 4  # score key-tiles per burst group (1 = no b-burst packing)


def _build(debug=False, gsz=None):
    gsz = GSZ if gsz is None else gsz
    import concourse.bacc as bacc
    import concourse.mybir as mybir
    from concourse.tile import TileContext

    dt = mybir.dt
    f32, f32r, bf16 = dt.float32, dt.float32r, dt.bfloat16
    AF = mybir.ActivationFunctionType

    nc = bacc.Bacc(
        "TRN2",
        target_bir_lowering=False,
        debug=debug,
        enable_asserts=False,
        num_devices=NCORES,
    )

    xT = nc.declare_dram_parameter("xT", [D, T], bf16, isOutput=False)
    wqT = nc.declare_dram_parameter("wqT", [D, EP], bf16, isOutput=False)
    wkT = nc.declare_dram_parameter("wkT", [D, EP], bf16, isOutput=False)
    wvT = nc.declare_dram_parameter("wvT", [D, DV], bf16, isOutput=False)
    woT = nc.declare_dram_parameter("woT", [DV, D], bf16, isOutput=False)
    cosN = nc.declare_dram_parameter("cosN", [T, HD], bf16, isOutput=False)
    identB = nc.declare_dram_parameter("identB", [128, 128], bf16, isOutput=False)
    sinN = nc.declare_dram_parameter("sinN", [T, HD], bf16, isOutput=False)
    out = nc.declare_dram_parameter("out", [T, D], f32, isOutput=True)

    def rope(qraw, qtl, cos3, sin3, tmps):
        """qraw [128, EP] f32 -> qtl [128, EP] bf16 with rotary applied.

        Column map: head h dim e<128 -> col 128h+e ; dim 128+j -> col 512+32h+j.
        rot_half partner: e<72 -> e+72 (sign -), e>=72 -> e-72 (sign +).
        cos3/sin3: [128, 4(bcast), 144] broadcast views of this t-tile's rows.
        Two full products m1=q*cos, m2=q*sin (4 ops), then 4 region combines.
        """
        qa = qraw[:, 0:512].rearrange("p (h e) -> p h e", h=HL)
        qb = qraw[:, 512:EP].rearrange("p (h e) -> p h e", h=HL)
        oa = qtl[:, 0:512].rearrange("p (h e) -> p h e", h=HL)
        ob = qtl[:, 512:EP].rearrange("p (h e) -> p h e", h=HL)
        m1, m2 = tmps
        m1a = m1[:, 0:512].rearrange("p (h e) -> p h e", h=HL)
        m1b = m1[:, 512:EP].rearrange("p (h e) -> p h e", h=HL)
        m2a = m2[:, 0:512].rearrange("p (h e) -> p h e", h=HL)
        m2b = m2[:, 512:EP].rearrange("p (h e) -> p h e", h=HL)
        v = nc.vector
        v.tensor_mul(m1a[:, :, 0:128], qa[:, :, 0:128], cos3[:, :, 0:128])
        v.tensor_mul(m1b[:, :, 0:16], qb[:, :, 0:16], cos3[:, :, 128:144])
        # m2[j] = q[j] * sin[partner(j)] so combines read m2 at the partner col
        v.tensor_mul(m2a[:, :, 0:56], qa[:, :, 0:56], sin3[:, :, 72:128])
        v.tensor_mul(m2a[:, :, 56:72], qa[:, :, 56:72], sin3[:, :, 128:144])
        v.tensor_mul(m2a[:, :, 72:128], qa[:, :, 72:128], sin3[:, :, 0:56])
        v.tensor_mul(m2b[:, :, 0:16], qb[:, :, 0:16], sin3[:, :, 56:72])
        # e in [0,56):  out = m1[e] - m2[e+72]
        v.tensor_sub(oa[:, :, 0:56], m1a[:, :, 0:56], m2a[:, :, 72:128])
        # e in [56,72): partner lives in the b block
        v.tensor_sub(oa[:, :, 56:72], m1a[:, :, 56:72], m2b[:, :, 0:16])
        # e in [72,128): out = m1[e] + m2[e-72]
        v.tensor_add(oa[:, :, 72:128], m1a[:, :, 72:128], m2a[:, :, 0:56])
        # e in [128,144): out = m1b + m2[56:72]
        v.tensor_add(ob[:, :, 0:16], m1b[:, :, 0:16], m2a[:, :, 56:72])
        # zero the 16 pad cols of each head's b-block
        v.memset(ob[:, :, 16:32], 0.0)

    with TileContext(nc) as tc:
        with tc.tile_pool(name="persist", bufs=1) as P0:
            ident_bf = P0.tile([128, 128], bf16, name="ident_bf", tag="ident_bf")
            nc.sync.dma_start(ident_bf[:], identB[:])

            qTa = [
                P0.tile([128, T], bf16, name=f"qTa{h}", tag=f"qTa{h}")
                for h in range(HL)
            ]
            kTa = [
                P0.tile([128, T], bf16, name=f"kTa{h}", tag=f"kTa{h}")
                for h in range(HL)
            ]
            qTB = P0.tile([128, T], bf16, name="qTB", tag="qTB")
            kTB = P0.tile([128, T], bf16, name="kTB", tag="kTB")
            # per-head zero-padded copies of kTB: kTBz[h] has head h's 16
            # tail rows (at 32h..32h+16) and ZEROS elsewhere.  The tail
            # score matmul then runs as a normal K=128 full-array matmul
            # (rhs = qTB, whose other heads' rows are nulled by the zero
            # weights) — avoiding the 32-row tiling-mode switch that drains
            # the PE array on every key tile.
            kTBz = [
                P0.tile([128, T], bf16, name=f"kTBz{h}", tag=f"kTBz{h}")
                for h in range(HL)
            ]
            vt = [
                P0.tile([128, HL * (HD + 1)], bf16, name=f"v{t}", tag=f"v{t}")
                for t in range(NT)
            ]

            # ---------------- Phase A: projections + rope + transposes -----
            with (
                tc.tile_pool(name="pa", bufs=1) as pa,
                tc.tile_pool(name="paps", bufs=1, space="PSUM") as paps,
            ):
                xtiles = [
                    pa.tile([128, T], bf16, name=f"xTs{k}", tag=f"xTs{k}")
                    for k in range(KC)
                ]
                cos_sb = pa.tile([128, NT * HD], bf16, name="cos_sb", tag="cos_sb")
                sin_sb = pa.tile([128, NT * HD], bf16, name="sin_sb", tag="sin_sb")

                def trig3(sb, n):
                    # [128, 144] row block for t-tile n, broadcast over 4 heads
                    return (
                        sb[:, n * HD : (n + 1) * HD]
                        .rearrange("p (o r) -> p o r", o=1)
                        .to_broadcast([128, HL, HD])
                    )

                def proj_phase(wdram, width, consume_head, consume_tail, first=False):
                    wtiles = []
                    for k in range(KC):
                        wt_ = pa.tile(
                            [128, EP], bf16, name=f"w{k}", tag=f"W{k}"
                        )
                        nsp = 2 if (first and k < 3) else 1
                        w_ = width // nsp
                        for j in range(nsp):
                            nc.sync.dma_start(
                                wt_[:, j * w_ : (j + 1) * w_],
                                wdram[k * 128 : (k + 1) * 128, j * w_ : (j + 1) * w_],
                            )
                        wtiles.append(wt_)
                        if first:
                            # interleave the x chunk right after its weight
                            # chunk so matmul k can start as soon as pair k
                            # lands, instead of waiting for the whole load
                            nsp = 8 if k == 0 else (4 if k < 3 else 2)
                            w_ = T // nsp
                            for j in range(nsp):
                                nc.sync.dma_start(
                                    xtiles[k][:, j * w_ : (j + 1) * w_],
                                    xT[
                                        k * 128 : (k + 1) * 128,
                                        j * w_ : (j + 1) * w_,
                                    ],
                                )
                    half = width // 2
                    pending = None
                    for n in range(NT):
                        ps0 = paps.tile([128, 320], f32, name="ps0", tag="proj", bufs=6)
                        ps1 = paps.tile([128, 320], f32, name="ps1", tag="proj", bufs=6)
                        for k in range(KC):
                            st, sp = k == 0, k == KC - 1
                            lhs = xtiles[k][:, n * 128 : (n + 1) * 128]
                            nc.tensor.matmul(
                                ps0[:, 0:half],
                                lhs,
                                wtiles[k][:, 0:half],
                                start=st,
                                stop=sp,
                            )
                            nc.tensor.matmul(
                                ps1[:, 0:half],
                                lhs,
                                wtiles[k][:, half:width],
                                start=st,
                                stop=sp,
                            )
                        if pending is not None:
                            consume_tail(*pending)
                            pending = None
                        carry = consume_head(n, ps0[:, 0:half], ps1[:, 0:half])
                        if consume_tail is not None:
                            pending = (n, carry)
                    if pending is not None:
                        consume_tail(*pending)

                def qk_consume(qtl_dst_a, qtl_dst_b):
                    def head(n, ps0, ps1):
                        qraw = pa.tile([128, EP], f32, name="qraw", tag="qraw", bufs=3)
                        nc.any.tensor_copy(qraw[:, 0:320], ps0)
                        nc.any.tensor_copy(qraw[:, 320:EP], ps1)
                        qtl = pa.tile([128, EP], bf16, name="qtl", tag="qtl", bufs=3)
                        tA = pa.tile([128, EP], f32, name="ropeA", tag="ropeA", bufs=2)
                        tB = pa.tile([128, EP], f32, name="ropeB", tag="ropeB", bufs=2)
                        rope(qraw, qtl, trig3(cos_sb, n), trig3(sin_sb, n), (tA, tB))
                        return qtl

                    def tail(n, qtl):
                        for j in range(5):
                            tp = paps.tile(
                                [128, 128], bf16, name="tp", tag="tp", bufs=2
                            )
                            nc.tensor.transpose(
                                tp[:], qtl[:, 128 * j : 128 * (j + 1)], ident_bf[:]
                            )
                            dst = qtl_dst_a[j] if j < 4 else qtl_dst_b
                            nc.any.tensor_copy(
                                dst[:, n * 128 : (n + 1) * 128], tp[:]
                            )

                    return head, tail

                def v_consume(n, ps0, ps1):
                    v3 = vt[n].rearrange("p (h e) -> p h e", h=HL)
                    nc.any.tensor_copy(
                        v3[:, 0:2, 0:HD],
                        ps0.rearrange("p (h e) -> p h e", h=2),
                    )
                    nc.any.tensor_copy(
                        v3[:, 2:4, 0:HD],
                        ps1.rearrange("p (h e) -> p h e", h=2),
                    )
                    nc.vector.memset(v3[:, :, HD : HD + 1], 1.0)

                qh, qt_ = qk_consume(qTa, qTB)
                kh, kt_ = qk_consume(kTa, kTB)
                proj_phase(wvT, DV, v_consume, None, first=True)
                nc.sync.dma_start(
                    cos_sb.rearrange("p (n r) -> p n r", n=NT),
                    cosN.rearrange("(n p) r -> p n r", p=128),
                )
                nc.sync.dma_start(
                    sin_sb.rearrange("p (n r) -> p n r", n=NT),
                    sinN.rearrange("(n p) r -> p n r", p=128),
                )
                for hh in range(HL):
                    nc.vector.memset(kTBz[hh][:], 0.0)
                proj_phase(wqT, EP, qh, qt_)
                proj_phase(wkT, EP, kh, kt_)
                # scatter each head's 16 real tail rows into its zero tile;
                # h-major so head 0 lands first
                for hh in range(HL):
                    nc.sync.dma_start(
                        kTBz[hh][32 * hh : 32 * hh + 16, :],
                        kTB[32 * hh : 32 * hh + 16, :],
                    )

            # ---------------- Phase B: attention --------------------------
            with tc.tile_pool(name="pb", bufs=1) as pb:
                ot = [
                    pb.tile([128, DV], bf16, name=f"o{t}", tag=f"o{t}")
                    for t in range(NT)
                ]
                with tc.tile_pool(name="pbps", bufs=1, space="PSUM") as pbps:
                    for qb in range(4):
                        for h in range(HL):
                            # pack the 4 q-tile accumulators into 2 PSUM banks:
                            # 3*145 fp32 = 1740B fits one 2KB bank
                            o_ps3 = pbps.tile(
                                [128, 3 * (HD + 1)], f32, name="o_ps3", tag="o3", bufs=2
                            )
                            o_ps1 = pbps.tile(
                                [128, HD + 1], f32, name="o_ps1", tag="o1", bufs=2
                            )
                            o_ps = [
                                o_ps3[:, 0 : HD + 1],
                                o_ps3[:, HD + 1 : 2 * (HD + 1)],
                                o_ps3[:, 2 * (HD + 1) : 3 * (HD + 1)],
                                o_ps1[:],
                            ]

                            def s_exp_group(g):
                                # 4 key-tiles per group, paired into 2-bank
                                # PSUM tiles.  Head (K=128) and zero-padded
                                # tail (also K=128, via kTBz) matmuls all run
                                # in the default 128x128 array mode — no
                                # tiling-mode switches, so LDWEIGHTS prefetch
                                # keeps the PE streaming back-to-back.
                                sps2 = [
                                    pbps.tile(
                                        [128, 1024], f32, name="sps", tag="sc", bufs=2
                                    )
                                    for _ in range(gsz // 2)
                                ]
                                for j in range(gsz):
                                    kt = gsz * g + j
                                    dst = sps2[j // 2][
                                        :, (j % 2) * 512 : (j % 2) * 512 + 512
                                    ]
                                    nc.tensor.matmul(
                                        dst,
                                        kTa[h][:, kt * 128 : (kt + 1) * 128],
                                        qTa[h][:, qb * 512 : (qb + 1) * 512],
                                        start=True,
                                        stop=False,
                                    )
                                    nc.tensor.matmul(
                                        dst,
                                        kTBz[h][:, kt * 128 : (kt + 1) * 128],
                                        qTB[:, qb * 512 : (qb + 1) * 512],
                                        start=False,
                                        stop=True,
                                    )
                                Es = []
                                for j2 in range(gsz // 2):
                                    E = pb.tile(
                                        [128, 1024], bf16, name="E", tag="E", bufs=4
                                    )
                                    nc.scalar.activation(
                                        E[:], sps2[j2][:], AF.Exp, scale=SCALE
                                    )
                                    Es.append(E[:, 0:512])
                                    Es.append(E[:, 512:1024])
                                return Es

                            def pv_group(g, Es):
                                for j in range(gsz):
                                    kt = gsz * g + j
                                    for qt in range(4):
                                        # start/stop are bank-granular: qt 0-2
                                        # share o_ps3's bank, so only the
                                        # first/last bank write carries them
                                        if qt < 3:
                                            st = kt == 0 and qt == 0
                                            sp = kt == NT - 1 and qt == 2
                                        else:
                                            st = kt == 0
                                            sp = kt == NT - 1
                                        nc.tensor.matmul(
                                            o_ps[qt][:],
                                            Es[j][:, qt * 128 : (qt + 1) * 128],
                                            vt[kt][:, (HD + 1) * h : (HD + 1) * (h + 1)],
                                            start=st,
                                            stop=sp,
                                        )

                            ngrp = NT // gsz
                            Eprev = s_exp_group(0)
                            for g in range(ngrp):
                                Enext = s_exp_group(g + 1) if g + 1 < ngrp else None
                                pv_group(g, Eprev)
                                Eprev = Enext
                            for qt in range(4):
                                t = qb * 4 + qt
                                r = pb.tile([128, 1], f32, name="r", tag="r", bufs=4)
                                nc.vector.reciprocal(r[:], o_ps[qt][:, HD : HD + 1])
                                nc.vector.tensor_scalar_mul(
                                    ot[t][:, HD * h : HD * (h + 1)],
                                    o_ps[qt][:, 0:HD],
                                    r[:],
                                )

                # ---------------- Phase C: o^T + final projection ----------
                oTa = [
                    pb.tile([128, T], bf16, name=f"oTa{j}", tag=f"oTa{j}")
                    for j in range(4)
                ]
                oTb = pb.tile([64, T], bf16, name="oTb", tag="oTb")
                wo_tiles = []
                for k in range(5):
                    rows = 128 if k < 4 else 64
                    wot_ = pb.tile([128, D], bf16, name=f"wo{k}", tag=f"wo{k}")
                    nc.sync.dma_start(
                        wot_[0:rows, :], woT[k * 128 : k * 128 + rows, :]
                    )
                    wo_tiles.append(wot_)
                with tc.tile_pool(name="pcps", bufs=1, space="PSUM") as pcps:

                    def o_transp(t):
                        for j in range(4):
                            tp = pcps.tile(
                                [128, 128], bf16, name="tpo", tag="otp", bufs=3
                            )
                            nc.tensor.transpose(
                                tp[:],
                                ot[t][:, 128 * j : 128 * (j + 1)],
                                ident_bf[:],
                            )
                            nc.any.tensor_copy(
                                oTa[j][:, t * 128 : (t + 1) * 128], tp[:]
                            )
                        tpb = pcps.tile([64, 128], bf16, name="tpb", tag="otp", bufs=3)
                        nc.tensor.transpose(
                            tpb[:],
                            ot[t][:, 512:DV],
                            ident_bf[:],
                        )
                        nc.any.tensor_copy(
                            oTb[:, t * 128 : (t + 1) * 128], tpb[:]
                        )

                    def final(t):
                        for j3 in range(3):
                            fps = pcps.tile([128, 384], f32, name="fps", tag="f", bufs=3)
                            for k in range(5):
                                lhs = (
                                    oTa[k][:, t * 128 : (t + 1) * 128]
                                    if k < 4
                                    else oTb[:, t * 128 : (t + 1) * 128]
                                )
                                nc.tensor.matmul(
                                    fps[:],
                                    lhs,
                                    wo_tiles[k][
                                        0 : (128 if k < 4 else 64),
                                        384 * j3 : 384 * (j3 + 1),
                                    ],
                                    start=(k == 0),
                                    stop=(k == 4),
                                )
                            fout = pb.tile(
                                [128, 384], f32, name="fout", tag="fout", bufs=4
                            )
                            nc.any.tensor_copy(fout[:], fps[:])
                            nc.sync.dma_start(
                                out[
                                    t * 128 : (t + 1) * 128,
                                    384 * j3 : 384 * (j3 + 1),
                                ],
                                fout[:],
                            )

                    o_transp(0)
                    for t in range(NT):
                        if t + 1 < NT:
                            o_transp(t + 1)
                        final(t)

    nc.compile()
    return nc


def get_nc(debug=False, gsz=None):
    key = (bool(debug), GSZ if gsz is None else gsz)
    if key not in _NC_CACHE:
        _NC_CACHE[key] = _build(debug, gsz)
    return _NC_CACHE[key]


def make_in_maps(x, cos, sin, Wq, Wk, Wv, Wo):
    import ml_dtypes

    x = np.asarray(x, np.float32)
    cos = np.asarray(cos, np.float32)
    sin = np.asarray(sin, np.float32)
    Wq, Wk, Wv, Wo = (np.asarray(w, np.float32) for w in (Wq, Wk, Wv, Wo))
    cos_bf = cos.astype(ml_dtypes.bfloat16)
    sin_bf = sin.astype(ml_dtypes.bfloat16)

    in_maps = []
    for c in range(NCORES):
        b, hg = divmod(c, 2)
        heads = [HL * hg + i for i in range(HL)]

        def qk_w(W):
            Wsel = np.zeros((EP, D), np.float32)
            for i, g in enumerate(heads):
                Wsel[128 * i : 128 * i + 128] = W[144 * g : 144 * g + 128]
                Wsel[512 + 32 * i : 512 + 32 * i + 16] = W[144 * g + 128 : 144 * g + 144]
            return np.ascontiguousarray(Wsel.T)

        wv_sel = np.concatenate([Wv[144 * g : 144 * g + 144] for g in heads], 0)
        wo_sel = np.concatenate([Wo[:, 144 * g : 144 * g + 144] for g in heads], 1)
        in_maps.append(
            {
                "xT": np.ascontiguousarray(x[b].T).astype(ml_dtypes.bfloat16),
                "wqT": qk_w(Wq).astype(ml_dtypes.bfloat16),
                "wkT": qk_w(Wk).astype(ml_dtypes.bfloat16),
                "wvT": np.ascontiguousarray(wv_sel.T).astype(ml_dtypes.bfloat16),
                "woT": np.ascontiguousarray(wo_sel.T).astype(ml_dtypes.bfloat16),
                "cosN": cos_bf,
                "sinN": sin_bf,
                "identB": np.eye(128, dtype=ml_dtypes.bfloat16),
            }
        )
    return in_maps


def kernel(x, cos, sin, Wq, Wk, Wv, Wo, _trace=False, _trace_kwargs=None):
    from concourse.bass_utils import run_bass_kernel_spmd

    nc = get_nc()
    in_maps = make_in_maps(x, cos, sin, Wq, Wk, Wv, Wo)
    res = run_bass_kernel_spmd(
        nc,
        in_maps,
        list(range(NCORES)),
        trace=_trace,
        **(_trace_kwargs or {}),
    )
    parts = [res.results[c]["out"] for c in range(NCORES)]
    outb = np.stack([parts[2 * b] + parts[2 * b + 1] for b in range(B)])
    if _trace:
        kernel.last_results = res
    return outb.astype(np.float32)



# revision 12
# speedup vs baseline: 1.2378x; 1.2378x over previous
"""Trainium2 Bass kernel for a fused multi-head attention block.

Reference computation (B=4, T=2048, D=1152, H=8, HD=144, full rotary):
    q,k,v = x@Wq.T, x@Wk.T, x@Wv.T   (per head)
    q,k   = rope(q, k, cos, sin)
    o     = softmax(q k^T / sqrt(HD)) v
    out   = o @ Wo.T
Sharding (8 cores): core c = (batch b = c//2, head-group hg = c%2).
Each core computes 4 heads of one batch and a partial output
out_part = o_local @ Wo[:, hg_cols].T ; host sums the two partials per batch.

Design notes (v2):
  * q/k are projected DIRECTLY into transposed layout qT/kT [head_dim, T]
    (weight chunk stationary, xT streaming) so no PE transposes are needed
    before the score matmuls.  v keeps the [T, head_dim] layout for PV.
  * rope in transposed layout: partner(d) = d+-72 is a PARTITION shift,
    done with SBUF->SBUF DMAs; cos/sin live in [dim, T] layout with the
    rotate-half sign folded into sin host-side.  3 tensor ops per block.
  * head_dim 144 = 128 (main block per head) + 16 (tail).  The 4 heads'
    tails are packed into one shared 128-row projection block (q rows
    0:64, k rows 64:128).  Score tail matmuls are ZERO-PADDED to K=128
    (kTBz[h]: only head h's 16 rows non-zero) so every matmul runs in the
    default 128x128 array mode -- tiling-mode switches drain the PE.
  * Scores are computed transposed (S^T [keys, q]) so PV needs no
    transpose; softmax denominator comes free via a ones column in v.
  * exp() without max-subtraction: |scores*scale| < ~6, safe in fp32.
  * Phase C: o^T via PE transposes, then out[t,e] accumulated K-outer so
    the oT stationaries' LDWEIGHTS hide under 3 matmuls each.
"""

import numpy as np

B, T, D, H = 4, 2048, 1152, 8
HL = 4              # heads per core
HD = 144            # head dim
DV = HL * HD        # 576, v/o width
NT = T // 128       # 16 t-tiles
KC = D // 128       # 9 contraction chunks
SCALE = float(HD) ** -0.5
NCORES = 8

_NC_CACHE = {}
GSZ = 4  # score key-tiles per burst group


def _build(debug=False, gsz=None):
    gsz = GSZ if gsz is None else gsz
    import concourse.bacc as bacc
    import concourse.mybir as mybir
    from concourse.tile import TileContext

    dt = mybir.dt
    f32, bf16 = dt.float32, dt.bfloat16
    AF = mybir.ActivationFunctionType

    nc = bacc.Bacc(
        "TRN2",
        target_bir_lowering=False,
        debug=debug,
        enable_asserts=False,
        num_devices=NCORES,
    )

    xT = nc.declare_dram_parameter("xT", [D, T], bf16, isOutput=False)
    wqM = nc.declare_dram_parameter("wqM", [D, 512], bf16, isOutput=False)
    wkM = nc.declare_dram_parameter("wkM", [D, 512], bf16, isOutput=False)
    wqkT = nc.declare_dram_parameter("wqkT", [D, 128], bf16, isOutput=False)
    wvT = nc.declare_dram_parameter("wvT", [D, DV], bf16, isOutput=False)
    woT = nc.declare_dram_parameter("woT", [DV, D], bf16, isOutput=False)
    cosmT = nc.declare_dram_parameter("cosmT", [128, T], bf16, isOutput=False)
    sinmT = nc.declare_dram_parameter("sinmT", [128, T], bf16, isOutput=False)
    costF = nc.declare_dram_parameter("costF", [128, T], bf16, isOutput=False)
    sintF = nc.declare_dram_parameter("sintF", [128, T], bf16, isOutput=False)
    identB = nc.declare_dram_parameter("identB", [128, 128], bf16, isOutput=False)
    out = nc.declare_dram_parameter("out", [T, D], f32, isOutput=True)

    with TileContext(nc) as tc:
        with tc.tile_pool(name="persist", bufs=1) as P0:
            ident_bf = P0.tile([128, 128], bf16, name="ident_bf", tag="ident_bf")
            nc.sync.dma_start(ident_bf[:], identB[:])

            qTa = [
                P0.tile([128, T], bf16, name=f"qTa{h}", tag=f"qTa{h}")
                for h in range(HL)
            ]
            kTa = [
                P0.tile([128, T], bf16, name=f"kTa{h}", tag=f"kTa{h}")
                for h in range(HL)
            ]
            # roped tails: rows 0:64 q (16h+j = head h dim 128+j),
            # rows 64:128 k
            qkTB = P0.tile([128, T], bf16, name="qkTB", tag="qkTB")
            # zero-padded per-head k-tail stationaries (rows 16h:16h+16)
            kTBz = [
                P0.tile([128, T], bf16, name=f"kTBz{h}", tag=f"kTBz{h}")
                for h in range(HL)
            ]
            vt = [
                P0.tile([128, HL * (HD + 1)], bf16, name=f"v{t}", tag=f"v{t}")
                for t in range(NT)
            ]

            # ---------------- Phase A: projections + rope ------------------
            with tc.tile_pool(name="pa", bufs=1) as pa:
                xtiles = [
                    pa.tile([128, T], bf16, name=f"xTs{k}", tag=f"xTs{k}")
                    for k in range(KC)
                ]
                cosm_sb = pa.tile([128, T], bf16, name="cosm", tag="cosm")
                sinm_sb = pa.tile([128, T], bf16, name="sinm", tag="sinm")
                cost_sb = pa.tile([128, T], bf16, name="cost", tag="cost")
                sint_sb = pa.tile([128, T], bf16, name="sint", tag="sint")
                for hh in range(HL):
                    nc.vector.memset(kTBz[hh][:], 0.0)

                # ---- V projection (x-chunk stationary, wv streaming) ----
                with (
                    tc.tile_pool(name="pav", bufs=1) as pav,
                    tc.tile_pool(name="pavps", bufs=1, space="PSUM") as pavps,
                ):
                    wv_t = []
                    for k in range(KC):
                        wt_ = pav.tile([128, DV], bf16, name=f"wv{k}", tag=f"wv{k}")
                        nc.sync.dma_start(wt_[:], wvT[k * 128 : (k + 1) * 128, :])
                        wv_t.append(wt_)
                        # interleave x chunk loads right after their weight
                        nsp = 8 if k == 0 else (4 if k < 3 else 2)
                        w_ = T // nsp
                        for j in range(nsp):
                            nc.sync.dma_start(
                                xtiles[k][:, j * w_ : (j + 1) * w_],
                                xT[k * 128 : (k + 1) * 128, j * w_ : (j + 1) * w_],
                            )
                    nc.sync.dma_start(cosm_sb[:], cosmT[:])
                    nc.sync.dma_start(sinm_sb[:], sinmT[:])
                    nc.sync.dma_start(cost_sb[:], costF[:])
                    nc.sync.dma_start(sint_sb[:], sintF[:])
                    for t in range(NT):
                        psV = pavps.tile(
                            [128, DV], f32, name="psV", tag="vps", bufs=2
                        )
                        for k in range(KC):
                            st, sp = k == 0, k == KC - 1
                            lhs = xtiles[k][:, t * 128 : (t + 1) * 128]
                            nc.tensor.matmul(
                                psV[:, 0:512], lhs, wv_t[k][:, 0:512],
                                start=st, stop=sp,
                            )
                            nc.tensor.matmul(
                                psV[:, 512:DV], lhs, wv_t[k][:, 512:DV],
                                start=st, stop=sp,
                            )
                        v3 = vt[t].rearrange("p (h e) -> p h e", h=HL)
                        nc.any.tensor_copy(
                            v3[:, :, 0:HD],
                            psV.rearrange("p (h e) -> p h e", h=HL),
                        )
                        nc.vector.memset(v3[:, :, HD : HD + 1], 1.0)

                # ---- q/k transposed projections (weight stationary) ----
                with (
                    tc.tile_pool(name="paq", bufs=1) as paq,
                    tc.tile_pool(name="paqps", bufs=1, space="PSUM") as paqps,
                ):
                    wqk_t = []
                    for k in range(KC):
                        wt_ = paq.tile(
                            [128, 128], bf16, name=f"wqk{k}", tag=f"wqk{k}"
                        )
                        nc.sync.dma_start(wt_[:], wqkT[k * 128 : (k + 1) * 128, :])
                        wqk_t.append(wt_)
                    wm_t = [
                        paq.tile([128, 512], bf16, name=f"wm{k}", tag=f"wm{k}")
                        for k in range(KC)
                    ]

                    def load_wm(wdram):
                        for k in range(KC):
                            nc.sync.dma_start(
                                wm_t[k][:], wdram[k * 128 : (k + 1) * 128, :]
                            )

                    def block_mm(stat_fn):
                        ps = paqps.tile(
                            [128, T], f32, name="psQ", tag="qkps", bufs=2
                        )
                        for k in range(KC):
                            st, sp = k == 0, k == KC - 1
                            stat = stat_fn(k)
                            for c4 in range(4):
                                nc.tensor.matmul(
                                    ps[:, c4 * 512 : (c4 + 1) * 512],
                                    stat,
                                    xtiles[k][:, c4 * 512 : (c4 + 1) * 512],
                                    start=st,
                                    stop=sp,
                                )
                        return ps

                    def evac(ps, dst):
                        # per-bank copies (cross-bank PSUM reads are slow)
                        for c4 in range(4):
                            nc.any.tensor_copy(
                                dst[:, c4 * 512 : (c4 + 1) * 512],
                                ps[:, c4 * 512 : (c4 + 1) * 512],
                            )

                    # tail block first: q tails (cols 0:64) + k tails (64:128)
                    ps = block_mm(lambda k: wqk_t[k][:])
                    tailraw = pa.tile([128, T], bf16, name="tailraw", tag="tailraw")
                    evac(ps, tailraw)

                    def main_blocks(dst_list, tail_part):
                        # tail_part: 0 for q (tailraw rows 0:64), 1 for k
                        for h in range(HL):
                            ps = block_mm(
                                lambda k: wm_t[k][:, 128 * h : 128 * (h + 1)]
                            )
                            raw = pa.tile(
                                [128, T], bf16, name="raw", tag="raw", bufs=2
                            )
                            evac(ps, raw)
                            sh = pa.tile(
                                [128, T], bf16, name="sh", tag="sh", bufs=2
                            )
                            tb = 64 * tail_part + 16 * h
                            nc.sync.dma_start(sh[0:56, :], raw[72:128, :])
                            nc.sync.dma_start(sh[56:72, :], tailraw[tb : tb + 16, :])
                            nc.sync.dma_start(sh[72:128, :], raw[0:56, :])
                            # stash rows 56:72 (partner of the tail dims)
                            nc.sync.dma_start(
                                tailsh[tb : tb + 16, :], raw[56:72, :]
                            )
                            m1 = pa.tile([128, T], bf16, name="m1", tag="m1", bufs=2)
                            m2 = pa.tile([128, T], bf16, name="m2", tag="m2", bufs=2)
                            nc.vector.tensor_mul(m1[:], raw[:], cosm_sb[:])
                            nc.vector.tensor_mul(m2[:], sh[:], sinm_sb[:])
                            nc.vector.tensor_add(dst_list[h][:], m1[:], m2[:])

                    tailsh = pa.tile([128, T], bf16, name="tailsh", tag="tailsh")
                    load_wm(wqM)
                    main_blocks(qTa, 0)
                    load_wm(wkM)
                    main_blocks(kTa, 1)

                    # tail rope (both q and k tails at once)
                    tm1 = pa.tile([128, T], bf16, name="tm1", tag="m1", bufs=2)
                    tm2 = pa.tile([128, T], bf16, name="tm2", tag="m2", bufs=2)
                    nc.vector.tensor_mul(tm1[:], tailraw[:], cost_sb[:])
                    nc.vector.tensor_mul(tm2[:], tailsh[:], sint_sb[:])
                    nc.vector.tensor_add(qkTB[:], tm1[:], tm2[:])
                    # scatter roped k-tails into the zero-padded stationaries
                    for hh in range(HL):
                        nc.sync.dma_start(
                            kTBz[hh][16 * hh : 16 * hh + 16, :],
                            qkTB[64 + 16 * hh : 64 + 16 * hh + 16, :],
                        )

            # ---------------- Phase B: attention --------------------------
            with tc.tile_pool(name="pb", bufs=1) as pb:
                ot = [
                    pb.tile([128, DV], bf16, name=f"o{t}", tag=f"o{t}")
                    for t in range(NT)
                ]
                with tc.tile_pool(name="pbps", bufs=1, space="PSUM") as pbps:
                    for qb in range(4):
                        for h in range(HL):
                            # pack the 4 q-tile accumulators into 2 PSUM banks:
                            # 3*145 fp32 = 1740B fits one 2KB bank
                            o_ps3 = pbps.tile(
                                [128, 3 * (HD + 1)], f32, name="o_ps3", tag="o3", bufs=2
                            )
                            o_ps1 = pbps.tile(
                                [128, HD + 1], f32, name="o_ps1", tag="o1", bufs=2
                            )
                            o_ps = [
                                o_ps3[:, 0 : HD + 1],
                                o_ps3[:, HD + 1 : 2 * (HD + 1)],
                                o_ps3[:, 2 * (HD + 1) : 3 * (HD + 1)],
                                o_ps1[:],
                            ]

                            def s_exp_group(g):
                                # 4 key-tiles per group, paired into 2-bank
                                # PSUM tiles; all matmuls K=128 (tail via
                                # zero-padded kTBz) -> no mode switches.
                                sps2 = [
                                    pbps.tile(
                                        [128, 1024], f32, name="sps", tag="sc", bufs=2
                                    )
                                    for _ in range(gsz // 2)
                                ]
                                for j in range(gsz):
                                    kt = gsz * g + j
                                    dst = sps2[j // 2][
                                        :, (j % 2) * 512 : (j % 2) * 512 + 512
                                    ]
                                    nc.tensor.matmul(
                                        dst,
                                        kTa[h][:, kt * 128 : (kt + 1) * 128],
                                        qTa[h][:, qb * 512 : (qb + 1) * 512],
                                        start=True,
                                        stop=False,
                                    )
                                    nc.tensor.matmul(
                                        dst,
                                        kTBz[h][:, kt * 128 : (kt + 1) * 128],
                                        qkTB[:, qb * 512 : (qb + 1) * 512],
                                        start=False,
                                        stop=True,
                                    )
                                Es = []
                                for j2 in range(gsz // 2):
                                    E = pb.tile(
                                        [128, 1024], bf16, name="E", tag="E", bufs=4
                                    )
                                    nc.scalar.activation(
                                        E[:], sps2[j2][:], AF.Exp, scale=SCALE
                                    )
                                    Es.append(E[:, 0:512])
                                    Es.append(E[:, 512:1024])
                                return Es

                            def pv_group(g, Es):
                                for j in range(gsz):
                                    kt = gsz * g + j
                                    for qt in range(4):
                                        # start/stop are bank-granular: qt 0-2
                                        # share o_ps3's bank
                                        if qt < 3:
                                            st = kt == 0 and qt == 0
                                            sp = kt == NT - 1 and qt == 2
                                        else:
                                            st = kt == 0
                                            sp = kt == NT - 1
                                        nc.tensor.matmul(
                                            o_ps[qt][:],
                                            Es[j][:, qt * 128 : (qt + 1) * 128],
                                            vt[kt][:, (HD + 1) * h : (HD + 1) * (h + 1)],
                                            start=st,
                                            stop=sp,
                                        )

                            ngrp = NT // gsz
                            Eprev = s_exp_group(0)
                            for g in range(ngrp):
                                Enext = s_exp_group(g + 1) if g + 1 < ngrp else None
                                pv_group(g, Eprev)
                                Eprev = Enext
                            for qt in range(4):
                                t = qb * 4 + qt
                                r = pb.tile([128, 1], f32, name="r", tag="r", bufs=4)
                                nc.vector.reciprocal(r[:], o_ps[qt][:, HD : HD + 1])
                                nc.vector.tensor_scalar_mul(
                                    ot[t][:, HD * h : HD * (h + 1)],
                                    o_ps[qt][:, 0:HD],
                                    r[:],
                                )

                # ---------------- Phase C: o^T + final projection ----------
                oTa = [
                    pb.tile([128, T], bf16, name=f"oTa{j}", tag=f"oTa{j}")
                    for j in range(4)
                ]
                oTb = pb.tile([128, T], bf16, name="oTb", tag="oTb")
                nc.vector.memset(oTb[64:128, :], 0.0)
                wo_tiles = []
                for k in range(5):
                    rows = 128 if k < 4 else 64
                    wot_ = pb.tile([128, D], bf16, name=f"wo{k}", tag=f"wo{k}")
                    nc.sync.dma_start(
                        wot_[0:rows, :], woT[k * 128 : k * 128 + rows, :]
                    )
                    if rows < 128:
                        nc.vector.memset(wot_[rows:128, :], 0.0)
                    wo_tiles.append(wot_)
                with tc.tile_pool(name="pcps", bufs=1, space="PSUM") as pcps:

                    def o_transp(t):
                        for j in range(4):
                            tp = pcps.tile(
                                [128, 128], bf16, name="tpo", tag="otp", bufs=2
                            )
                            nc.tensor.transpose(
                                tp[:],
                                ot[t][:, 128 * j : 128 * (j + 1)],
                                ident_bf[:],
                            )
                            nc.any.tensor_copy(
                                oTa[j][:, t * 128 : (t + 1) * 128], tp[:]
                            )
                        tpb = pcps.tile([64, 128], bf16, name="tpb", tag="otp", bufs=2)
                        nc.tensor.transpose(
                            tpb[:],
                            ot[t][:, 512:DV],
                            ident_bf[:],
                        )
                        nc.any.tensor_copy(
                            oTb[0:64, t * 128 : (t + 1) * 128], tpb[:]
                        )

                    def final(t):
                        # K-outer: each oT stationary's LDWEIGHTS hides
                        # under the previous chunk's 3 matmuls
                        fps3 = [
                            pcps.tile(
                                [128, 384], f32, name=f"fps{j3}", tag=f"f{j3}", bufs=2
                            )
                            for j3 in range(3)
                        ]
                        for k in range(5):
                            lhs = (
                                oTa[k][:, t * 128 : (t + 1) * 128]
                                if k < 4
                                else oTb[:, t * 128 : (t + 1) * 128]
                            )
                            for j3 in range(3):
                                nc.tensor.matmul(
                                    fps3[j3][:],
                                    lhs,
                                    wo_tiles[k][:, 384 * j3 : 384 * (j3 + 1)],
                                    start=(k == 0),
                                    stop=(k == 4),
                                )
                        for j3 in range(3):
                            fout = pb.tile(
                                [128, 384], f32, name="fout", tag="fout", bufs=4
                            )
                            nc.any.tensor_copy(fout[:], fps3[j3][:])
                            nc.sync.dma_start(
                                out[
                                    t * 128 : (t + 1) * 128,
                                    384 * j3 : 384 * (j3 + 1),
                                ],
                                fout[:],
                            )

                    o_transp(0)
                    for t in range(NT):
                        if t + 1 < NT:
                            o_transp(t + 1)
                        final(t)

    nc.compile()
    return nc


def get_nc(debug=False, gsz=None):
    key = (bool(debug), GSZ if gsz is None else gsz)
    if key not in _NC_CACHE:
        _NC_CACHE[key] = _build(debug, gsz)
    return _NC_CACHE[key]


def make_in_maps(x, cos, sin, Wq, Wk, Wv, Wo):
    import ml_dtypes

    bf = ml_dtypes.bfloat16
    x = np.asarray(x, np.float32)
    cos = np.asarray(cos, np.float32)
    sin = np.asarray(sin, np.float32)
    Wq, Wk, Wv, Wo = (np.asarray(w, np.float32) for w in (Wq, Wk, Wv, Wo))

    cosT = cos.T  # [144, T]
    sinT = sin.T
    sign = np.where(np.arange(128) < 72, -1.0, 1.0).astype(np.float32)
    cosmT = np.ascontiguousarray(cosT[0:128]).astype(bf)
    sinmT = np.ascontiguousarray(sinT[0:128] * sign[:, None]).astype(bf)
    tidx = 128 + (np.arange(128) % 16)
    costF = np.ascontiguousarray(cosT[tidx]).astype(bf)
    sintF = np.ascontiguousarray(sinT[tidx]).astype(bf)

    in_maps = []
    for c in range(NCORES):
        b, hg = divmod(c, 2)
        heads = [HL * hg + i for i in range(HL)]

        def main_w(W):
            sel = np.concatenate(
                [W[144 * g : 144 * g + 128] for g in heads], 0
            )  # [512, D]
            return np.ascontiguousarray(sel.T).astype(bf)

        qk_tail = np.zeros((128, D), np.float32)
        for i, g in enumerate(heads):
            qk_tail[16 * i : 16 * i + 16] = Wq[144 * g + 128 : 144 * g + 144]
            qk_tail[64 + 16 * i : 64 + 16 * i + 16] = Wk[144 * g + 128 : 144 * g + 144]

        wv_sel = np.concatenate([Wv[144 * g : 144 * g + 144] for g in heads], 0)
        wo_sel = np.concatenate([Wo[:, 144 * g : 144 * g + 144] for g in heads], 1)
        in_maps.append(
            {
                "xT": np.ascontiguousarray(x[b].T).astype(bf),
                "wqM": main_w(Wq),
                "wkM": main_w(Wk),
                "wqkT": np.ascontiguousarray(qk_tail.T).astype(bf),
                "wvT": np.ascontiguousarray(wv_sel.T).astype(bf),
                "woT": np.ascontiguousarray(wo_sel.T).astype(bf),
                "cosmT": cosmT,
                "sinmT": sinmT,
                "costF": costF,
                "sintF": sintF,
                "identB": np.eye(128, dtype=bf),
            }
        )
    return in_maps


def kernel(x, cos, sin, Wq, Wk, Wv, Wo, _trace=False, _trace_kwargs=None):
    from concourse.bass_utils import run_bass_kernel_spmd

    nc = get_nc()
    in_maps = make_in_maps(x, cos, sin, Wq, Wk, Wv, Wo)
    res = run_bass_kernel_spmd(
        nc,
        in_maps,
        list(range(NCORES)),
        trace=_trace,
        **(_trace_kwargs or {}),
    )
    parts = [res.results[c]["out"] for c in range(NCORES)]
    outb = np.stack([parts[2 * b] + parts[2 * b + 1] for b in range(B)])
    if _trace:
        kernel.last_results = res
    return outb.astype(np.float32)


# revision 15
# speedup vs baseline: 1.2520x; 1.0115x over previous
"""Trainium2 Bass kernel for a fused multi-head attention block.

Reference computation (B=4, T=2048, D=1152, H=8, HD=144, full rotary):
    q,k,v = x@Wq.T, x@Wk.T, x@Wv.T   (per head)
    q,k   = rope(q, k, cos, sin)
    o     = softmax(q k^T / sqrt(HD)) v
    out   = o @ Wo.T
Sharding (8 cores): core c = (batch b = c//2, head-group hg = c%2).
Each core computes 4 heads of one batch and a partial output
out_part = o_local @ Wo[:, hg_cols].T ; host sums the two partials per batch.

Design notes (v2):
  * q/k are projected DIRECTLY into transposed layout qT/kT [head_dim, T]
    (weight chunk stationary, xT streaming) so no PE transposes are needed
    before the score matmuls.  v keeps the [T, head_dim] layout for PV.
  * rope in transposed layout: partner(d) = d+-72 is a PARTITION shift,
    done with SBUF->SBUF DMAs; cos/sin live in [dim, T] layout with the
    rotate-half sign folded into sin host-side.  3 tensor ops per block.
  * head_dim 144 = 128 (main block per head) + 16 (tail).  The 4 heads'
    tails are packed into one shared 128-row projection block (q rows
    0:64, k rows 64:128).  Score tail matmuls are ZERO-PADDED to K=128
    (kTBz[h]: only head h's 16 rows non-zero) so every matmul runs in the
    default 128x128 array mode -- tiling-mode switches drain the PE.
  * Scores are computed transposed (S^T [keys, q]) so PV needs no
    transpose; softmax denominator comes free via a ones column in v.
  * exp() without max-subtraction: |scores*scale| < ~6, safe in fp32.
  * Phase C: o^T via PE transposes, then out[t,e] accumulated K-outer so
    the oT stationaries' LDWEIGHTS hide under 3 matmuls each.
"""

import numpy as np

B, T, D, H = 4, 2048, 1152, 8
HL = 4              # heads per core
HD = 144            # head dim
DV = HL * HD        # 576, v/o width
NT = T // 128       # 16 t-tiles
KC = D // 128       # 9 contraction chunks
SCALE = float(HD) ** -0.5
NCORES = 8

_NC_CACHE = {}
GSZ = 4  # score key-tiles per burst group


def _build(debug=False, gsz=None):
    gsz = GSZ if gsz is None else gsz
    import concourse.bacc as bacc
    import concourse.mybir as mybir
    from concourse.tile import TileContext

    dt = mybir.dt
    f32, bf16 = dt.float32, dt.bfloat16
    AF = mybir.ActivationFunctionType

    nc = bacc.Bacc(
        "TRN2",
        target_bir_lowering=False,
        debug=debug,
        enable_asserts=False,
        num_devices=NCORES,
    )

    xT = nc.declare_dram_parameter("xT", [D, T], bf16, isOutput=False)
    wqM = nc.declare_dram_parameter("wqM", [D, 512], bf16, isOutput=False)
    wkM = nc.declare_dram_parameter("wkM", [D, 512], bf16, isOutput=False)
    wqkT = nc.declare_dram_parameter("wqkT", [D, 128], bf16, isOutput=False)
    wvT = nc.declare_dram_parameter("wvT", [D, DV], bf16, isOutput=False)
    woT = nc.declare_dram_parameter("woT", [DV, D], bf16, isOutput=False)
    cosmT = nc.declare_dram_parameter("cosmT", [128, T], bf16, isOutput=False)
    sinmT = nc.declare_dram_parameter("sinmT", [128, T], bf16, isOutput=False)
    costF = nc.declare_dram_parameter("costF", [128, T], bf16, isOutput=False)
    sintF = nc.declare_dram_parameter("sintF", [128, T], bf16, isOutput=False)
    identB = nc.declare_dram_parameter("identB", [128, 128], bf16, isOutput=False)
    out = nc.declare_dram_parameter("out", [T, D], f32, isOutput=True)

    with TileContext(nc) as tc:
        with tc.tile_pool(name="persist", bufs=1) as P0:
            ident_bf = P0.tile([128, 128], bf16, name="ident_bf", tag="ident_bf")
            nc.sync.dma_start(ident_bf[:], identB[:])

            qTa = [
                P0.tile([128, T], bf16, name=f"qTa{h}", tag=f"qTa{h}")
                for h in range(HL)
            ]
            kTa = [
                P0.tile([128, T], bf16, name=f"kTa{h}", tag=f"kTa{h}")
                for h in range(HL)
            ]
            # roped tails: rows 0:64 q (16h+j = head h dim 128+j),
            # rows 64:128 k
            qkTB = P0.tile([128, T], bf16, name="qkTB", tag="qkTB")
            # zero-padded per-head k-tail stationaries (rows 16h:16h+16)
            kTBz = [
                P0.tile([128, T], bf16, name=f"kTBz{h}", tag=f"kTBz{h}")
                for h in range(HL)
            ]
            vt = [
                P0.tile([128, HL * (HD + 1)], bf16, name=f"v{t}", tag=f"v{t}")
                for t in range(NT)
            ]

            # ---------------- Phase A: projections + rope ------------------
            with tc.tile_pool(name="pa", bufs=1) as pa:
                xtiles = [
                    pa.tile([128, T], bf16, name=f"xTs{k}", tag=f"xTs{k}")
                    for k in range(KC)
                ]
                cosm_sb = pa.tile([128, T], bf16, name="cosm", tag="cosm")
                sinm_sb = pa.tile([128, T], bf16, name="sinm", tag="sinm")
                cost_sb = pa.tile([128, T], bf16, name="cost", tag="cost")
                sint_sb = pa.tile([128, T], bf16, name="sint", tag="sint")
                for hh in range(HL):
                    nc.vector.memset(kTBz[hh][:], 0.0)

                # ---- q/k transposed projections (weight stationary), then
                # ---- V last so its rope-independent matmuls keep the PE
                # ---- busy while the tail rope + kTBz scatter complete.
                with (
                    tc.tile_pool(name="paq", bufs=1) as paq,
                    tc.tile_pool(name="paqps", bufs=1, space="PSUM") as paqps,
                ):
                    wqk_t = []
                    for k in range(KC):
                        wt_ = paq.tile(
                            [128, 128], bf16, name=f"wqk{k}", tag=f"wqk{k}"
                        )
                        nc.sync.dma_start(wt_[:], wqkT[k * 128 : (k + 1) * 128, :])
                        wqk_t.append(wt_)
                        # interleave x chunk loads right after their weight
                        nsp = 8 if k == 0 else (4 if k < 3 else 2)
                        w_ = T // nsp
                        for j in range(nsp):
                            nc.sync.dma_start(
                                xtiles[k][:, j * w_ : (j + 1) * w_],
                                xT[k * 128 : (k + 1) * 128, j * w_ : (j + 1) * w_],
                            )
                    wm_t = [
                        paq.tile([128, 512], bf16, name=f"wm{k}", tag=f"wm{k}")
                        for k in range(KC)
                    ]

                    def load_wm(wdram):
                        for k in range(KC):
                            nc.sync.dma_start(
                                wm_t[k][:], wdram[k * 128 : (k + 1) * 128, :]
                            )

                    load_wm(wkM)
                    nc.sync.dma_start(cosm_sb[:], cosmT[:])
                    nc.sync.dma_start(sinm_sb[:], sinmT[:])
                    nc.sync.dma_start(cost_sb[:], costF[:])
                    nc.sync.dma_start(sint_sb[:], sintF[:])

                    def block_mm(stat_fn):
                        ps = paqps.tile(
                            [128, T], f32, name="psQ", tag="qkps", bufs=2
                        )
                        for k in range(KC):
                            st, sp = k == 0, k == KC - 1
                            stat = stat_fn(k)
                            for c4 in range(4):
                                nc.tensor.matmul(
                                    ps[:, c4 * 512 : (c4 + 1) * 512],
                                    stat,
                                    xtiles[k][:, c4 * 512 : (c4 + 1) * 512],
                                    start=st,
                                    stop=sp,
                                )
                        return ps

                    def evac(ps, dst):
                        # per-bank copies (cross-bank PSUM reads are slow)
                        for c4 in range(4):
                            nc.any.tensor_copy(
                                dst[:, c4 * 512 : (c4 + 1) * 512],
                                ps[:, c4 * 512 : (c4 + 1) * 512],
                            )

                    # tail block first: q tails (cols 0:64) + k tails (64:128)
                    ps = block_mm(lambda k: wqk_t[k][:])
                    tailraw = pa.tile([128, T], bf16, name="tailraw", tag="tailraw")
                    evac(ps, tailraw)
                    tailsh = pa.tile([128, T], bf16, name="tailsh", tag="tailsh")
                    # tail cos-product is ready as soon as tailraw lands
                    tm1 = pa.tile([128, T], bf16, name="tm1", tag="tm1")
                    nc.vector.tensor_mul(tm1[:], tailraw[:], cost_sb[:])

                    def main_blocks(dst_list, tail_part):
                        # tail_part: 0 for q (tailraw rows 0:64), 1 for k
                        for h in range(HL):
                            ps = block_mm(
                                lambda k: wm_t[k][:, 128 * h : 128 * (h + 1)]
                            )
                            raw = pa.tile(
                                [128, T], bf16, name="raw", tag="raw", bufs=2
                            )
                            evac(ps, raw)
                            sh = pa.tile(
                                [128, T], bf16, name="sh", tag="sh", bufs=2
                            )
                            tb = 64 * tail_part + 16 * h
                            nc.sync.dma_start(sh[0:56, :], raw[72:128, :])
                            nc.sync.dma_start(sh[56:72, :], tailraw[tb : tb + 16, :])
                            nc.sync.dma_start(sh[72:128, :], raw[0:56, :])
                            # stash rows 56:72 (partner of the tail dims)
                            nc.sync.dma_start(
                                tailsh[tb : tb + 16, :], raw[56:72, :]
                            )
                            m1 = pa.tile([128, T], bf16, name="m1", tag="m1", bufs=2)
                            m2 = pa.tile([128, T], bf16, name="m2", tag="m2", bufs=2)
                            nc.vector.tensor_mul(m1[:], raw[:], cosm_sb[:])
                            nc.vector.tensor_mul(m2[:], sh[:], sinm_sb[:])
                            nc.vector.tensor_add(dst_list[h][:], m1[:], m2[:])

                        # rope this half of the tail block (k half unblocks
                        # the kTBz scatter long before the q mains finish);
                        # slices keep all operands at the same base partition
                        lo = 64 * tail_part
                        tm2 = pa.tile([128, T], bf16, name="tm2", tag="m2", bufs=2)
                        nc.vector.tensor_mul(
                            tm2[lo : lo + 64, :],
                            tailsh[lo : lo + 64, :],
                            sint_sb[lo : lo + 64, :],
                        )
                        nc.vector.tensor_add(
                            qkTB[lo : lo + 64, :],
                            tm1[lo : lo + 64, :],
                            tm2[lo : lo + 64, :],
                        )
                        if tail_part == 1:
                            for hh in range(HL):
                                nc.sync.dma_start(
                                    kTBz[hh][16 * hh : 16 * hh + 16, :],
                                    qkTB[64 + 16 * hh : 64 + 16 * hh + 16, :],
                                )

                    main_blocks(kTa, 1)
                    load_wm(wqM)
                    main_blocks(qTa, 0)

                # ---- V projection (x-chunk stationary, wv streaming) ----
                with (
                    tc.tile_pool(name="pav", bufs=1) as pav,
                    tc.tile_pool(name="pavps", bufs=1, space="PSUM") as pavps,
                ):
                    wv_t = []
                    for k in range(KC):
                        wt_ = pav.tile([128, DV], bf16, name=f"wv{k}", tag=f"wv{k}")
                        nc.sync.dma_start(wt_[:], wvT[k * 128 : (k + 1) * 128, :])
                        wv_t.append(wt_)
                    for t in range(NT):
                        psV = pavps.tile(
                            [128, DV], f32, name="psV", tag="vps", bufs=2
                        )
                        for k in range(KC):
                            st, sp = k == 0, k == KC - 1
                            lhs = xtiles[k][:, t * 128 : (t + 1) * 128]
                            nc.tensor.matmul(
                                psV[:, 0:512], lhs, wv_t[k][:, 0:512],
                                start=st, stop=sp,
                            )
                            nc.tensor.matmul(
                                psV[:, 512:DV], lhs, wv_t[k][:, 512:DV],
                                start=st, stop=sp,
                            )
                        v3 = vt[t].rearrange("p (h e) -> p h e", h=HL)
                        nc.any.tensor_copy(
                            v3[:, :, 0:HD],
                            psV.rearrange("p (h e) -> p h e", h=HL),
                        )
                        nc.vector.memset(v3[:, :, HD : HD + 1], 1.0)

            # ---------------- Phase B: attention --------------------------
            with tc.tile_pool(name="pb", bufs=1) as pb:
                ot = [
                    pb.tile([128, DV], bf16, name=f"o{t}", tag=f"o{t}")
                    for t in range(NT)
                ]
                with tc.tile_pool(name="pbps", bufs=1, space="PSUM") as pbps:
                    for qb in range(4):
                        for h in range(HL):
                            # pack the 4 q-tile accumulators into 2 PSUM banks:
                            # 3*145 fp32 = 1740B fits one 2KB bank
                            o_ps3 = pbps.tile(
                                [128, 3 * (HD + 1)], f32, name="o_ps3", tag="o3", bufs=2
                            )
                            o_ps1 = pbps.tile(
                                [128, HD + 1], f32, name="o_ps1", tag="o1", bufs=2
                            )
                            o_ps = [
                                o_ps3[:, 0 : HD + 1],
                                o_ps3[:, HD + 1 : 2 * (HD + 1)],
                                o_ps3[:, 2 * (HD + 1) : 3 * (HD + 1)],
                                o_ps1[:],
                            ]

                            def s_exp_group(g):
                                # 4 key-tiles per group, paired into 2-bank
                                # PSUM tiles; all matmuls K=128 (tail via
                                # zero-padded kTBz) -> no mode switches.
                                sps2 = [
                                    pbps.tile(
                                        [128, 1024], f32, name="sps", tag="sc", bufs=2
                                    )
                                    for _ in range(gsz // 2)
                                ]
                                for j in range(gsz):
                                    kt = gsz * g + j
                                    dst = sps2[j // 2][
                                        :, (j % 2) * 512 : (j % 2) * 512 + 512
                                    ]
                                    nc.tensor.matmul(
                                        dst,
                                        kTa[h][:, kt * 128 : (kt + 1) * 128],
                                        qTa[h][:, qb * 512 : (qb + 1) * 512],
                                        start=True,
                                        stop=False,
                                    )
                                for j in range(gsz):
                                    kt = gsz * g + j
                                    dst = sps2[j // 2][
                                        :, (j % 2) * 512 : (j % 2) * 512 + 512
                                    ]
                                    nc.tensor.matmul(
                                        dst,
                                        kTBz[h][:, kt * 128 : (kt + 1) * 128],
                                        qkTB[:, qb * 512 : (qb + 1) * 512],
                                        start=False,
                                        stop=True,
                                    )
                                Es = []
                                for j2 in range(gsz // 2):
                                    E = pb.tile(
                                        [128, 1024], bf16, name="E", tag="E", bufs=4
                                    )
                                    nc.scalar.activation(
                                        E[:], sps2[j2][:], AF.Exp, scale=SCALE
                                    )
                                    Es.append(E[:, 0:512])
                                    Es.append(E[:, 512:1024])
                                return Es

                            def pv_group(g, Es):
                                for j in range(gsz):
                                    kt = gsz * g + j
                                    for qt in range(4):
                                        # start/stop are bank-granular: qt 0-2
                                        # share o_ps3's bank
                                        if qt < 3:
                                            st = kt == 0 and qt == 0
                                            sp = kt == NT - 1 and qt == 2
                                        else:
                                            st = kt == 0
                                            sp = kt == NT - 1
                                        nc.tensor.matmul(
                                            o_ps[qt][:],
                                            Es[j][:, qt * 128 : (qt + 1) * 128],
                                            vt[kt][:, (HD + 1) * h : (HD + 1) * (h + 1)],
                                            start=st,
                                            stop=sp,
                                        )

                            ngrp = NT // gsz
                            Eprev = s_exp_group(0)
                            for g in range(ngrp):
                                Enext = s_exp_group(g + 1) if g + 1 < ngrp else None
                                pv_group(g, Eprev)
                                Eprev = Enext
                            for qt in range(4):
                                t = qb * 4 + qt
                                r = pb.tile([128, 1], f32, name="r", tag="r", bufs=4)
                                nc.vector.reciprocal(r[:], o_ps[qt][:, HD : HD + 1])
                                nc.vector.tensor_scalar_mul(
                                    ot[t][:, HD * h : HD * (h + 1)],
                                    o_ps[qt][:, 0:HD],
                                    r[:],
                                )

                # ---------------- Phase C: o^T + final projection ----------
                oTa = [
                    pb.tile([128, T], bf16, name=f"oTa{j}", tag=f"oTa{j}")
                    for j in range(4)
                ]
                oTb = pb.tile([128, T], bf16, name="oTb", tag="oTb")
                nc.vector.memset(oTb[64:128, :], 0.0)
                wo_tiles = []
                for k in range(5):
                    rows = 128 if k < 4 else 64
                    wot_ = pb.tile([128, D], bf16, name=f"wo{k}", tag=f"wo{k}")
                    nc.sync.dma_start(
                        wot_[0:rows, :], woT[k * 128 : k * 128 + rows, :]
                    )
                    if rows < 128:
                        nc.vector.memset(wot_[rows:128, :], 0.0)
                    wo_tiles.append(wot_)
                with tc.tile_pool(name="pcps", bufs=1, space="PSUM") as pcps:

                    def o_transp(t):
                        for j in range(4):
                            tp = pcps.tile(
                                [128, 128], bf16, name="tpo", tag="otp", bufs=2
                            )
                            nc.tensor.transpose(
                                tp[:],
                                ot[t][:, 128 * j : 128 * (j + 1)],
                                ident_bf[:],
                            )
                            nc.any.tensor_copy(
                                oTa[j][:, t * 128 : (t + 1) * 128], tp[:]
                            )
                        tpb = pcps.tile([64, 128], bf16, name="tpb", tag="otp", bufs=2)
                        nc.tensor.transpose(
                            tpb[:],
                            ot[t][:, 512:DV],
                            ident_bf[:],
                        )
                        nc.any.tensor_copy(
                            oTb[0:64, t * 128 : (t + 1) * 128], tpb[:]
                        )

                    def final(t):
                        # K-outer: each oT stationary's LDWEIGHTS hides
                        # under the previous chunk's 3 matmuls
                        fps3 = [
                            pcps.tile(
                                [128, 384], f32, name=f"fps{j3}", tag=f"f{j3}", bufs=2
                            )
                            for j3 in range(3)
                        ]
                        for k in range(5):
                            lhs = (
                                oTa[k][:, t * 128 : (t + 1) * 128]
                                if k < 4
                                else oTb[:, t * 128 : (t + 1) * 128]
                            )
                            for j3 in range(3):
                                nc.tensor.matmul(
                                    fps3[j3][:],
                                    lhs,
                                    wo_tiles[k][:, 384 * j3 : 384 * (j3 + 1)],
                                    start=(k == 0),
                                    stop=(k == 4),
                                )
                        for j3 in range(3):
                            fout = pb.tile(
                                [128, 384], f32, name="fout", tag="fout", bufs=4
                            )
                            nc.any.tensor_copy(fout[:], fps3[j3][:])
                            nc.sync.dma_start(
                                out[
                                    t * 128 : (t + 1) * 128,
                                    384 * j3 : 384 * (j3 + 1),
                                ],
                                fout[:],
                            )

                    o_transp(0)
                    for t in range(NT):
                        if t + 1 < NT:
                            o_transp(t + 1)
                        final(t)

    nc.compile()
    return nc


def get_nc(debug=False, gsz=None):
    key = (bool(debug), GSZ if gsz is None else gsz)
    if key not in _NC_CACHE:
        _NC_CACHE[key] = _build(debug, gsz)
    return _NC_CACHE[key]


def make_in_maps(x, cos, sin, Wq, Wk, Wv, Wo):
    import ml_dtypes

    bf = ml_dtypes.bfloat16
    x = np.asarray(x, np.float32)
    cos = np.asarray(cos, np.float32)
    sin = np.asarray(sin, np.float32)
    Wq, Wk, Wv, Wo = (np.asarray(w, np.float32) for w in (Wq, Wk, Wv, Wo))

    cosT = cos.T  # [144, T]
    sinT = sin.T
    sign = np.where(np.arange(128) < 72, -1.0, 1.0).astype(np.float32)
    cosmT = np.ascontiguousarray(cosT[0:128]).astype(bf)
    sinmT = np.ascontiguousarray(sinT[0:128] * sign[:, None]).astype(bf)
    tidx = 128 + (np.arange(128) % 16)
    costF = np.ascontiguousarray(cosT[tidx]).astype(bf)
    sintF = np.ascontiguousarray(sinT[tidx]).astype(bf)

    in_maps = []
    for c in range(NCORES):
        b, hg = divmod(c, 2)
        heads = [HL * hg + i for i in range(HL)]

        def main_w(W):
            sel = np.concatenate(
                [W[144 * g : 144 * g + 128] for g in heads], 0
            )  # [512, D]
            return np.ascontiguousarray(sel.T).astype(bf)

        qk_tail = np.zeros((128, D), np.float32)
        for i, g in enumerate(heads):
            qk_tail[16 * i : 16 * i + 16] = Wq[144 * g + 128 : 144 * g + 144]
            qk_tail[64 + 16 * i : 64 + 16 * i + 16] = Wk[144 * g + 128 : 144 * g + 144]

        wv_sel = np.concatenate([Wv[144 * g : 144 * g + 144] for g in heads], 0)
        wo_sel = np.concatenate([Wo[:, 144 * g : 144 * g + 144] for g in heads], 1)
        in_maps.append(
            {
                "xT": np.ascontiguousarray(x[b].T).astype(bf),
                "wqM": main_w(Wq),
                "wkM": main_w(Wk),
                "wqkT": np.ascontiguousarray(qk_tail.T).astype(bf),
                "wvT": np.ascontiguousarray(wv_sel.T).astype(bf),
                "woT": np.ascontiguousarray(wo_sel.T).astype(bf),
                "cosmT": cosmT,
                "sinmT": sinmT,
                "costF": costF,
                "sintF": sintF,
                "identB": np.eye(128, dtype=bf),
            }
        )
    return in_maps


def kernel(x, cos, sin, Wq, Wk, Wv, Wo, _trace=False, _trace_kwargs=None):
    from concourse.bass_utils import run_bass_kernel_spmd

    nc = get_nc()
    in_maps = make_in_maps(x, cos, sin, Wq, Wk, Wv, Wo)
    res = run_bass_kernel_spmd(
        nc,
        in_maps,
        list(range(NCORES)),
        trace=_trace,
        **(_trace_kwargs or {}),
    )
    parts = [res.results[c]["out"] for c in range(NCORES)]
    outb = np.stack([parts[2 * b] + parts[2 * b + 1] for b in range(B)])
    if _trace:
        kernel.last_results = res
    return outb.astype(np.float32)


# revision 16
# speedup vs baseline: 1.3009x; 1.0390x over previous
"""Trainium2 Bass kernel for a fused multi-head attention block.

Reference computation (B=4, T=2048, D=1152, H=8, HD=144, full rotary):
    q,k,v = x@Wq.T, x@Wk.T, x@Wv.T   (per head)
    q,k   = rope(q, k, cos, sin)
    o     = softmax(q k^T / sqrt(HD)) v
    out   = o @ Wo.T
Sharding (8 cores): core c = (batch b = c//2, head-group hg = c%2).
Each core computes 4 heads of one batch and a partial output
out_part = o_local @ Wo[:, hg_cols].T ; host sums the two partials per batch.

Design notes (v2):
  * q/k are projected DIRECTLY into transposed layout qT/kT [head_dim, T]
    (weight chunk stationary, xT streaming) so no PE transposes are needed
    before the score matmuls.  v keeps the [T, head_dim] layout for PV.
  * rope in transposed layout: partner(d) = d+-72 is a PARTITION shift,
    done with SBUF->SBUF DMAs; cos/sin live in [dim, T] layout with the
    rotate-half sign folded into sin host-side.  3 tensor ops per block.
  * head_dim 144 = 128 (main block per head) + 16 (tail).  The 4 heads'
    tails are packed into one shared 128-row projection block (q rows
    0:64, k rows 64:128).  Score tail matmuls are ZERO-PADDED to K=128
    (kTBz[h]: only head h's 16 rows non-zero) so every matmul runs in the
    default 128x128 array mode -- tiling-mode switches drain the PE.
  * Scores are computed transposed (S^T [keys, q]) so PV needs no
    transpose; softmax denominator comes free via a ones column in v.
  * exp() without max-subtraction: |scores*scale| < ~6, safe in fp32.
  * Phase C: o^T via PE transposes, then out[t,e] accumulated K-outer so
    the oT stationaries' LDWEIGHTS hide under 3 matmuls each.
"""

import numpy as np

B, T, D, H = 4, 2048, 1152, 8
HL = 4              # heads per core
HD = 144            # head dim
DV = HL * HD        # 576, v/o width
NT = T // 128       # 16 t-tiles
KC = D // 128       # 9 contraction chunks
SCALE = float(HD) ** -0.5
NCORES = 8

_NC_CACHE = {}
GSZ = 4  # score key-tiles per burst group


def _build(debug=False, gsz=None):
    gsz = GSZ if gsz is None else gsz
    import concourse.bacc as bacc
    import concourse.mybir as mybir
    from concourse.tile import TileContext

    dt = mybir.dt
    f32, bf16 = dt.float32, dt.bfloat16
    AF = mybir.ActivationFunctionType

    nc = bacc.Bacc(
        "TRN2",
        target_bir_lowering=False,
        debug=debug,
        enable_asserts=False,
        num_devices=NCORES,
    )

    xT = nc.declare_dram_parameter("xT", [D, T], bf16, isOutput=False)
    wqM = nc.declare_dram_parameter("wqM", [D, 512], bf16, isOutput=False)
    wkM = nc.declare_dram_parameter("wkM", [D, 512], bf16, isOutput=False)
    wqkT = nc.declare_dram_parameter("wqkT", [D, 128], bf16, isOutput=False)
    wvT = nc.declare_dram_parameter("wvT", [D, DV], bf16, isOutput=False)
    woT = nc.declare_dram_parameter("woT", [DV, D], bf16, isOutput=False)
    cosmT = nc.declare_dram_parameter("cosmT", [128, T], bf16, isOutput=False)
    sinmT = nc.declare_dram_parameter("sinmT", [128, T], bf16, isOutput=False)
    costF = nc.declare_dram_parameter("costF", [128, T], bf16, isOutput=False)
    sintF = nc.declare_dram_parameter("sintF", [128, T], bf16, isOutput=False)
    identB = nc.declare_dram_parameter("identB", [128, 128], bf16, isOutput=False)
    out = nc.declare_dram_parameter("out", [T, D], f32, isOutput=True)

    with TileContext(nc) as tc:
        with tc.tile_pool(name="persist", bufs=1) as P0:
            ident_bf = P0.tile([128, 128], bf16, name="ident_bf", tag="ident_bf")
            nc.sync.dma_start(ident_bf[:], identB[:])

            qTa = [
                P0.tile([128, T], bf16, name=f"qTa{h}", tag=f"qTa{h}")
                for h in range(HL)
            ]
            kTa = [
                P0.tile([128, T], bf16, name=f"kTa{h}", tag=f"kTa{h}")
                for h in range(HL)
            ]
            # roped tails: rows 0:64 q (16h+j = head h dim 128+j),
            # rows 64:128 k
            qkTB = P0.tile([128, T], bf16, name="qkTB", tag="qkTB")
            # zero-padded per-head k-tail stationaries (rows 16h:16h+16)
            kTBz = [
                P0.tile([128, T], bf16, name=f"kTBz{h}", tag=f"kTBz{h}")
                for h in range(HL)
            ]
            vt = [
                P0.tile([128, HL * (HD + 1)], bf16, name=f"v{t}", tag=f"v{t}")
                for t in range(NT)
            ]

            # ---------------- Phase A: projections + rope ------------------
            with tc.tile_pool(name="pa", bufs=1) as pa:
                xtiles = [
                    pa.tile([128, T], bf16, name=f"xTs{k}", tag=f"xTs{k}")
                    for k in range(KC)
                ]
                cosm_sb = pa.tile([128, T], bf16, name="cosm", tag="cosm")
                sinm_sb = pa.tile([128, T], bf16, name="sinm", tag="sinm")
                cost_sb = pa.tile([128, T], bf16, name="cost", tag="cost")
                sint_sb = pa.tile([128, T], bf16, name="sint", tag="sint")
                for hh in range(HL):
                    nc.vector.memset(kTBz[hh][:], 0.0)

                # ---- q/k transposed projections (weight stationary), then
                # ---- V last so its rope-independent matmuls keep the PE
                # ---- busy while the tail rope + kTBz scatter complete.
                with (
                    tc.tile_pool(name="paq", bufs=1) as paq,
                    tc.tile_pool(name="paqps", bufs=1, space="PSUM") as paqps,
                ):
                    wqk_t = []
                    for k in range(KC):
                        wt_ = paq.tile(
                            [128, 128], bf16, name=f"wqk{k}", tag=f"wqk{k}"
                        )
                        nc.sync.dma_start(wt_[:], wqkT[k * 128 : (k + 1) * 128, :])
                        wqk_t.append(wt_)
                        # interleave x chunk loads right after their weight
                        nsp = 8 if k == 0 else (4 if k < 3 else 2)
                        w_ = T // nsp
                        for j in range(nsp):
                            nc.sync.dma_start(
                                xtiles[k][:, j * w_ : (j + 1) * w_],
                                xT[k * 128 : (k + 1) * 128, j * w_ : (j + 1) * w_],
                            )
                    wm_t = [
                        paq.tile([128, 512], bf16, name=f"wm{k}", tag=f"wm{k}")
                        for k in range(KC)
                    ]

                    def load_wm(wdram):
                        for k in range(KC):
                            nc.sync.dma_start(
                                wm_t[k][:], wdram[k * 128 : (k + 1) * 128, :]
                            )

                    load_wm(wkM)
                    nc.sync.dma_start(cosm_sb[:], cosmT[:])
                    nc.sync.dma_start(sinm_sb[:], sinmT[:])
                    nc.sync.dma_start(cost_sb[:], costF[:])
                    nc.sync.dma_start(sint_sb[:], sintF[:])

                    def block_mm(stat_fn):
                        ps = paqps.tile(
                            [128, T], f32, name="psQ", tag="qkps", bufs=2
                        )
                        for k in range(KC):
                            st, sp = k == 0, k == KC - 1
                            stat = stat_fn(k)
                            for c4 in range(4):
                                nc.tensor.matmul(
                                    ps[:, c4 * 512 : (c4 + 1) * 512],
                                    stat,
                                    xtiles[k][:, c4 * 512 : (c4 + 1) * 512],
                                    start=st,
                                    stop=sp,
                                )
                        return ps

                    def evac(ps, dst):
                        # per-bank copies (cross-bank PSUM reads are slow)
                        for c4 in range(4):
                            nc.any.tensor_copy(
                                dst[:, c4 * 512 : (c4 + 1) * 512],
                                ps[:, c4 * 512 : (c4 + 1) * 512],
                            )

                    # tail block first: q tails (cols 0:64) + k tails (64:128)
                    ps = block_mm(lambda k: wqk_t[k][:])
                    tailraw = pa.tile([128, T], bf16, name="tailraw", tag="tailraw")
                    evac(ps, tailraw)
                    tailsh = pa.tile([128, T], bf16, name="tailsh", tag="tailsh")
                    # tail cos-product is ready as soon as tailraw lands
                    tm1 = pa.tile([128, T], bf16, name="tm1", tag="tm1")
                    nc.vector.tensor_mul(tm1[:], tailraw[:], cost_sb[:])

                    def main_blocks(dst_list, tail_part):
                        # tail_part: 0 for q (tailraw rows 0:64), 1 for k
                        for h in range(HL):
                            ps = block_mm(
                                lambda k: wm_t[k][:, 128 * h : 128 * (h + 1)]
                            )
                            raw = pa.tile(
                                [128, T], bf16, name="raw", tag="raw", bufs=2
                            )
                            evac(ps, raw)
                            sh = pa.tile(
                                [128, T], bf16, name="sh", tag="sh", bufs=2
                            )
                            tb = 64 * tail_part + 16 * h
                            nc.gpsimd.dma_start(sh[0:56, :], raw[72:128, :])
                            nc.gpsimd.dma_start(sh[56:72, :], tailraw[tb : tb + 16, :])
                            nc.gpsimd.dma_start(sh[72:128, :], raw[0:56, :])
                            # stash rows 56:72 (partner of the tail dims)
                            nc.gpsimd.dma_start(
                                tailsh[tb : tb + 16, :], raw[56:72, :]
                            )
                            m1 = pa.tile([128, T], bf16, name="m1", tag="m1", bufs=2)
                            m2 = pa.tile([128, T], bf16, name="m2", tag="m2", bufs=2)
                            nc.vector.tensor_mul(m1[:], raw[:], cosm_sb[:])
                            nc.vector.tensor_mul(m2[:], sh[:], sinm_sb[:])
                            nc.vector.tensor_add(dst_list[h][:], m1[:], m2[:])

                        # rope this half of the tail block (k half unblocks
                        # the kTBz scatter long before the q mains finish);
                        # slices keep all operands at the same base partition
                        lo = 64 * tail_part
                        tm2 = pa.tile([128, T], bf16, name="tm2", tag="m2", bufs=2)
                        nc.vector.tensor_mul(
                            tm2[lo : lo + 64, :],
                            tailsh[lo : lo + 64, :],
                            sint_sb[lo : lo + 64, :],
                        )
                        nc.vector.tensor_add(
                            qkTB[lo : lo + 64, :],
                            tm1[lo : lo + 64, :],
                            tm2[lo : lo + 64, :],
                        )
                        if tail_part == 1:
                            for hh in range(HL):
                                nc.gpsimd.dma_start(
                                    kTBz[hh][16 * hh : 16 * hh + 16, :],
                                    qkTB[64 + 16 * hh : 64 + 16 * hh + 16, :],
                                )

                    main_blocks(kTa, 1)
                    load_wm(wqM)
                    main_blocks(qTa, 0)

                # ---- V projection (x-chunk stationary, wv streaming) ----
                with (
                    tc.tile_pool(name="pav", bufs=1) as pav,
                    tc.tile_pool(name="pavps", bufs=1, space="PSUM") as pavps,
                ):
                    wv_t = []
                    for k in range(KC):
                        wt_ = pav.tile([128, DV], bf16, name=f"wv{k}", tag=f"wv{k}")
                        nc.sync.dma_start(wt_[:], wvT[k * 128 : (k + 1) * 128, :])
                        wv_t.append(wt_)
                    for t in range(NT):
                        psV = pavps.tile(
                            [128, DV], f32, name="psV", tag="vps", bufs=2
                        )
                        for k in range(KC):
                            st, sp = k == 0, k == KC - 1
                            lhs = xtiles[k][:, t * 128 : (t + 1) * 128]
                            nc.tensor.matmul(
                                psV[:, 0:512], lhs, wv_t[k][:, 0:512],
                                start=st, stop=sp,
                            )
                            nc.tensor.matmul(
                                psV[:, 512:DV], lhs, wv_t[k][:, 512:DV],
                                start=st, stop=sp,
                            )
                        v3 = vt[t].rearrange("p (h e) -> p h e", h=HL)
                        nc.any.tensor_copy(
                            v3[:, :, 0:HD],
                            psV.rearrange("p (h e) -> p h e", h=HL),
                        )
                        nc.vector.memset(v3[:, :, HD : HD + 1], 1.0)

            # ---------------- Phase B: attention --------------------------
            with tc.tile_pool(name="pb", bufs=1) as pb:
                ot = [
                    pb.tile([128, DV], bf16, name=f"o{t}", tag=f"o{t}")
                    for t in range(NT)
                ]
                with tc.tile_pool(name="pbps", bufs=1, space="PSUM") as pbps:
                    for qb in range(4):
                        for h in range(HL):
                            # pack the 4 q-tile accumulators into 2 PSUM banks:
                            # 3*145 fp32 = 1740B fits one 2KB bank
                            o_ps3 = pbps.tile(
                                [128, 3 * (HD + 1)], f32, name="o_ps3", tag="o3", bufs=1
                            )
                            o_ps1 = pbps.tile(
                                [128, HD + 1], f32, name="o_ps1", tag="o1", bufs=1
                            )
                            o_ps = [
                                o_ps3[:, 0 : HD + 1],
                                o_ps3[:, HD + 1 : 2 * (HD + 1)],
                                o_ps3[:, 2 * (HD + 1) : 3 * (HD + 1)],
                                o_ps1[:],
                            ]

                            def s_exp_group(g):
                                # 4 key-tiles per group, paired into 2-bank
                                # PSUM tiles; all matmuls K=128 (tail via
                                # zero-padded kTBz) -> no mode switches.
                                sps2 = [
                                    pbps.tile(
                                        [128, 1024], f32, name="sps", tag="sc", bufs=3
                                    )
                                    for _ in range(gsz // 2)
                                ]
                                for j in range(gsz):
                                    kt = gsz * g + j
                                    dst = sps2[j // 2][
                                        :, (j % 2) * 512 : (j % 2) * 512 + 512
                                    ]
                                    nc.tensor.matmul(
                                        dst,
                                        kTa[h][:, kt * 128 : (kt + 1) * 128],
                                        qTa[h][:, qb * 512 : (qb + 1) * 512],
                                        start=True,
                                        stop=False,
                                    )
                                for j in range(gsz):
                                    kt = gsz * g + j
                                    dst = sps2[j // 2][
                                        :, (j % 2) * 512 : (j % 2) * 512 + 512
                                    ]
                                    nc.tensor.matmul(
                                        dst,
                                        kTBz[h][:, kt * 128 : (kt + 1) * 128],
                                        qkTB[:, qb * 512 : (qb + 1) * 512],
                                        start=False,
                                        stop=True,
                                    )
                                Es = []
                                for j2 in range(gsz // 2):
                                    E = pb.tile(
                                        [128, 1024], bf16, name="E", tag="E", bufs=4
                                    )
                                    nc.scalar.activation(
                                        E[:], sps2[j2][:], AF.Exp, scale=SCALE
                                    )
                                    Es.append(E[:, 0:512])
                                    Es.append(E[:, 512:1024])
                                return Es

                            def pv_group(g, Es):
                                for j in range(gsz):
                                    kt = gsz * g + j
                                    for qt in range(4):
                                        # start/stop are bank-granular: qt 0-2
                                        # share o_ps3's bank
                                        if qt < 3:
                                            st = kt == 0 and qt == 0
                                            sp = kt == NT - 1 and qt == 2
                                        else:
                                            st = kt == 0
                                            sp = kt == NT - 1
                                        nc.tensor.matmul(
                                            o_ps[qt][:],
                                            Es[j][:, qt * 128 : (qt + 1) * 128],
                                            vt[kt][:, (HD + 1) * h : (HD + 1) * (h + 1)],
                                            start=st,
                                            stop=sp,
                                        )

                            ngrp = NT // gsz
                            Eprev = s_exp_group(0)
                            for g in range(ngrp):
                                Enext = s_exp_group(g + 1) if g + 1 < ngrp else None
                                pv_group(g, Eprev)
                                Eprev = Enext
                            for qt in range(4):
                                t = qb * 4 + qt
                                r = pb.tile([128, 1], f32, name="r", tag="r", bufs=4)
                                nc.vector.reciprocal(r[:], o_ps[qt][:, HD : HD + 1])
                                nc.vector.tensor_scalar_mul(
                                    ot[t][:, HD * h : HD * (h + 1)],
                                    o_ps[qt][:, 0:HD],
                                    r[:],
                                )

                # ---------------- Phase C: o^T + final projection ----------
                oTa = [
                    pb.tile([128, T], bf16, name=f"oTa{j}", tag=f"oTa{j}")
                    for j in range(4)
                ]
                oTb = pb.tile([128, T], bf16, name="oTb", tag="oTb")
                nc.vector.memset(oTb[64:128, :], 0.0)
                wo_tiles = []
                for k in range(5):
                    rows = 128 if k < 4 else 64
                    wot_ = pb.tile([128, D], bf16, name=f"wo{k}", tag=f"wo{k}")
                    nc.sync.dma_start(
                        wot_[0:rows, :], woT[k * 128 : k * 128 + rows, :]
                    )
                    if rows < 128:
                        nc.vector.memset(wot_[rows:128, :], 0.0)
                    wo_tiles.append(wot_)
                with tc.tile_pool(name="pcps", bufs=1, space="PSUM") as pcps:

                    def o_transp(t):
                        for j in range(4):
                            tp = pcps.tile(
                                [128, 128], bf16, name="tpo", tag="otp", bufs=2
                            )
                            nc.tensor.transpose(
                                tp[:],
                                ot[t][:, 128 * j : 128 * (j + 1)],
                                ident_bf[:],
                            )
                            nc.any.tensor_copy(
                                oTa[j][:, t * 128 : (t + 1) * 128], tp[:]
                            )
                        tpb = pcps.tile([64, 128], bf16, name="tpb", tag="otp", bufs=2)
                        nc.tensor.transpose(
                            tpb[:],
                            ot[t][:, 512:DV],
                            ident_bf[:],
                        )
                        nc.any.tensor_copy(
                            oTb[0:64, t * 128 : (t + 1) * 128], tpb[:]
                        )

                    def final(t):
                        # K-outer: each oT stationary's LDWEIGHTS hides
                        # under the previous chunk's 3 matmuls
                        fps3 = [
                            pcps.tile(
                                [128, 384], f32, name=f"fps{j3}", tag=f"f{j3}", bufs=2
                            )
                            for j3 in range(3)
                        ]
                        for k in range(5):
                            lhs = (
                                oTa[k][:, t * 128 : (t + 1) * 128]
                                if k < 4
                                else oTb[:, t * 128 : (t + 1) * 128]
                            )
                            for j3 in range(3):
                                nc.tensor.matmul(
                                    fps3[j3][:],
                                    lhs,
                                    wo_tiles[k][:, 384 * j3 : 384 * (j3 + 1)],
                                    start=(k == 0),
                                    stop=(k == 4),
                                )
                        for j3 in range(3):
                            fout = pb.tile(
                                [128, 384], f32, name="fout", tag="fout", bufs=4
                            )
                            nc.any.tensor_copy(fout[:], fps3[j3][:])
                            nc.sync.dma_start(
                                out[
                                    t * 128 : (t + 1) * 128,
                                    384 * j3 : 384 * (j3 + 1),
                                ],
                                fout[:],
                            )

                    o_transp(0)
                    for t in range(NT):
                        if t + 1 < NT:
                            o_transp(t + 1)
                        final(t)

    nc.compile()
    return nc


def get_nc(debug=False, gsz=None):
    key = (bool(debug), GSZ if gsz is None else gsz)
    if key not in _NC_CACHE:
        _NC_CACHE[key] = _build(debug, gsz)
    return _NC_CACHE[key]


def make_in_maps(x, cos, sin, Wq, Wk, Wv, Wo):
    import ml_dtypes

    bf = ml_dtypes.bfloat16
    x = np.asarray(x, np.float32)
    cos = np.asarray(cos, np.float32)
    sin = np.asarray(sin, np.float32)
    Wq, Wk, Wv, Wo = (np.asarray(w, np.float32) for w in (Wq, Wk, Wv, Wo))

    cosT = cos.T  # [144, T]
    sinT = sin.T
    sign = np.where(np.arange(128) < 72, -1.0, 1.0).astype(np.float32)
    cosmT = np.ascontiguousarray(cosT[0:128]).astype(bf)
    sinmT = np.ascontiguousarray(sinT[0:128] * sign[:, None]).astype(bf)
    tidx = 128 + (np.arange(128) % 16)
    costF = np.ascontiguousarray(cosT[tidx]).astype(bf)
    sintF = np.ascontiguousarray(sinT[tidx]).astype(bf)

    in_maps = []
    for c in range(NCORES):
        b, hg = divmod(c, 2)
        heads = [HL * hg + i for i in range(HL)]

        def main_w(W):
            sel = np.concatenate(
                [W[144 * g : 144 * g + 128] for g in heads], 0
            )  # [512, D]
            return np.ascontiguousarray(sel.T).astype(bf)

        qk_tail = np.zeros((128, D), np.float32)
        for i, g in enumerate(heads):
            qk_tail[16 * i : 16 * i + 16] = Wq[144 * g + 128 : 144 * g + 144]
            qk_tail[64 + 16 * i : 64 + 16 * i + 16] = Wk[144 * g + 128 : 144 * g + 144]

        wv_sel = np.concatenate([Wv[144 * g : 144 * g + 144] for g in heads], 0)
        wo_sel = np.concatenate([Wo[:, 144 * g : 144 * g + 144] for g in heads], 1)
        in_maps.append(
            {
                "xT": np.ascontiguousarray(x[b].T).astype(bf),
                "wqM": main_w(Wq),
                "wkM": main_w(Wk),
                "wqkT": np.ascontiguousarray(qk_tail.T).astype(bf),
                "wvT": np.ascontiguousarray(wv_sel.T).astype(bf),
                "woT": np.ascontiguousarray(wo_sel.T).astype(bf),
                "cosmT": cosmT,
                "sinmT": sinmT,
                "costF": costF,
                "sintF": sintF,
                "identB": np.eye(128, dtype=bf),
            }
        )
    return in_maps


def kernel(x, cos, sin, Wq, Wk, Wv, Wo, _trace=False, _trace_kwargs=None):
    from concourse.bass_utils import run_bass_kernel_spmd

    nc = get_nc()
    in_maps = make_in_maps(x, cos, sin, Wq, Wk, Wv, Wo)
    res = run_bass_kernel_spmd(
        nc,
        in_maps,
        list(range(NCORES)),
        trace=_trace,
        **(_trace_kwargs or {}),
    )
    parts = [res.results[c]["out"] for c in range(NCORES)]
    outb = np.stack([parts[2 * b] + parts[2 * b + 1] for b in range(B)])
    if _trace:
        kernel.last_results = res
    return outb.astype(np.float32)


# revision 18
# speedup vs baseline: 1.3087x; 1.0060x over previous
"""Trainium2 Bass kernel for a fused multi-head attention block.

Reference computation (B=4, T=2048, D=1152, H=8, HD=144, full rotary):
    q,k,v = x@Wq.T, x@Wk.T, x@Wv.T   (per head)
    q,k   = rope(q, k, cos, sin)
    o     = softmax(q k^T / sqrt(HD)) v
    out   = o @ Wo.T
Sharding (8 cores): core c = (batch b = c//2, head-group hg = c%2).
Each core computes 4 heads of one batch and a partial output
out_part = o_local @ Wo[:, hg_cols].T ; host sums the two partials per batch.

Design notes (v2):
  * q/k are projected DIRECTLY into transposed layout qT/kT [head_dim, T]
    (weight chunk stationary, xT streaming) so no PE transposes are needed
    before the score matmuls.  v keeps the [T, head_dim] layout for PV.
  * rope in transposed layout: partner(d) = d+-72 is a PARTITION shift,
    done with SBUF->SBUF DMAs; cos/sin live in [dim, T] layout with the
    rotate-half sign folded into sin host-side.  3 tensor ops per block.
  * head_dim 144 = 128 (main block per head) + 16 (tail).  The 4 heads'
    tails are packed into one shared 128-row projection block (q rows
    0:64, k rows 64:128).  Score tail matmuls are ZERO-PADDED to K=128
    (kTBz[h]: only head h's 16 rows non-zero) so every matmul runs in the
    default 128x128 array mode -- tiling-mode switches drain the PE.
  * Scores are computed transposed (S^T [keys, q]) so PV needs no
    transpose; softmax denominator comes free via a ones column in v.
  * exp() without max-subtraction: |scores*scale| < ~6, safe in fp32.
  * Phase C: o^T via PE transposes, then out[t,e] accumulated K-outer so
    the oT stationaries' LDWEIGHTS hide under 3 matmuls each.
"""

import numpy as np

B, T, D, H = 4, 2048, 1152, 8
HL = 4              # heads per core
HD = 144            # head dim
DV = HL * HD        # 576, v/o width
NT = T // 128       # 16 t-tiles
KC = D // 128       # 9 contraction chunks
SCALE = float(HD) ** -0.5
NCORES = 8

_NC_CACHE = {}
GSZ = 4  # score key-tiles per burst group


def _build(debug=False, gsz=None):
    gsz = GSZ if gsz is None else gsz
    import concourse.bacc as bacc
    import concourse.mybir as mybir
    from concourse.tile import TileContext

    dt = mybir.dt
    f32, bf16 = dt.float32, dt.bfloat16
    AF = mybir.ActivationFunctionType

    nc = bacc.Bacc(
        "TRN2",
        target_bir_lowering=False,
        debug=debug,
        enable_asserts=False,
        num_devices=NCORES,
    )

    xT = nc.declare_dram_parameter("xT", [D, T], bf16, isOutput=False)
    wqM = nc.declare_dram_parameter("wqM", [D, 512], bf16, isOutput=False)
    wkM = nc.declare_dram_parameter("wkM", [D, 512], bf16, isOutput=False)
    wqkT = nc.declare_dram_parameter("wqkT", [D, 128], bf16, isOutput=False)
    wvT = nc.declare_dram_parameter("wvT", [D, DV], bf16, isOutput=False)
    woT = nc.declare_dram_parameter("woT", [DV, D], bf16, isOutput=False)
    cosmT = nc.declare_dram_parameter("cosmT", [128, T], bf16, isOutput=False)
    sinmT = nc.declare_dram_parameter("sinmT", [128, T], bf16, isOutput=False)
    costF = nc.declare_dram_parameter("costF", [128, T], bf16, isOutput=False)
    sintF = nc.declare_dram_parameter("sintF", [128, T], bf16, isOutput=False)
    identB = nc.declare_dram_parameter("identB", [128, 128], bf16, isOutput=False)
    out = nc.declare_dram_parameter("out", [T, D], f32, isOutput=True)

    with TileContext(nc) as tc:
        with tc.tile_pool(name="persist", bufs=1) as P0:
            ident_bf = P0.tile([128, 128], bf16, name="ident_bf", tag="ident_bf")
            nc.sync.dma_start(ident_bf[:], identB[:])

            qTa = [
                P0.tile([128, T], bf16, name=f"qTa{h}", tag=f"qTa{h}")
                for h in range(HL)
            ]
            kTa = [
                P0.tile([128, T], bf16, name=f"kTa{h}", tag=f"kTa{h}")
                for h in range(HL)
            ]
            # roped tails: rows 0:64 q (16h+j = head h dim 128+j),
            # rows 64:128 k
            qkTB = P0.tile([128, T], bf16, name="qkTB", tag="qkTB")
            # zero-padded per-head k-tail stationaries (rows 16h:16h+16)
            kTBz = [
                P0.tile([128, T], bf16, name=f"kTBz{h}", tag=f"kTBz{h}")
                for h in range(HL)
            ]
            vt = [
                P0.tile([128, HL * (HD + 1)], bf16, name=f"v{t}", tag=f"v{t}")
                for t in range(NT)
            ]

            # ---------------- Phase A: projections + rope ------------------
            with tc.tile_pool(name="pa", bufs=1) as pa:
                xtiles = [
                    pa.tile([128, T], bf16, name=f"xTs{k}", tag=f"xTs{k}")
                    for k in range(KC)
                ]
                cosm_sb = pa.tile([128, T], bf16, name="cosm", tag="cosm")
                sinm_sb = pa.tile([128, T], bf16, name="sinm", tag="sinm")
                cost_sb = pa.tile([128, T], bf16, name="cost", tag="cost")
                sint_sb = pa.tile([128, T], bf16, name="sint", tag="sint")
                for hh in range(HL):
                    nc.vector.memset(kTBz[hh][:], 0.0)

                # ---- q/k transposed projections (weight stationary), then
                # ---- V last so its rope-independent matmuls keep the PE
                # ---- busy while the tail rope + kTBz scatter complete.
                with (
                    tc.tile_pool(name="paq", bufs=1) as paq,
                    tc.tile_pool(name="paqps", bufs=1, space="PSUM") as paqps,
                ):
                    wqk_t = []
                    for k in range(KC):
                        wt_ = paq.tile(
                            [128, 128], bf16, name=f"wqk{k}", tag=f"wqk{k}"
                        )
                        nc.sync.dma_start(wt_[:], wqkT[k * 128 : (k + 1) * 128, :])
                        wqk_t.append(wt_)
                    wm_t = [
                        paq.tile([128, 512], bf16, name=f"wm{k}", tag=f"wm{k}")
                        for k in range(KC)
                    ]
                    for k in range(KC):
                        # interleave x chunk + k-main weight loads so the
                        # interleaved first block pair never starves
                        nc.sync.dma_start(
                            wm_t[k][:], wkM[k * 128 : (k + 1) * 128, :]
                        )
                        nsp = 8 if k == 0 else (4 if k < 3 else 2)
                        w_ = T // nsp
                        for j in range(nsp):
                            nc.sync.dma_start(
                                xtiles[k][:, j * w_ : (j + 1) * w_],
                                xT[k * 128 : (k + 1) * 128, j * w_ : (j + 1) * w_],
                            )

                    def load_wm(wdram):
                        for k in range(KC):
                            nc.sync.dma_start(
                                wm_t[k][:], wdram[k * 128 : (k + 1) * 128, :]
                            )
                    nc.sync.dma_start(cosm_sb[:], cosmT[:])
                    nc.sync.dma_start(sinm_sb[:], sinmT[:])
                    nc.sync.dma_start(cost_sb[:], costF[:])
                    nc.sync.dma_start(sint_sb[:], sintF[:])

                    def block_mm(stat_fn):
                        ps = paqps.tile(
                            [128, T], f32, name="psQ", tag="qkps", bufs=2
                        )
                        for k in range(KC):
                            st, sp = k == 0, k == KC - 1
                            stat = stat_fn(k)
                            for c4 in range(4):
                                nc.tensor.matmul(
                                    ps[:, c4 * 512 : (c4 + 1) * 512],
                                    stat,
                                    xtiles[k][:, c4 * 512 : (c4 + 1) * 512],
                                    start=st,
                                    stop=sp,
                                )
                        return ps

                    def evac(ps, dst):
                        # per-bank copies (cross-bank PSUM reads are slow)
                        for c4 in range(4):
                            nc.any.tensor_copy(
                                dst[:, c4 * 512 : (c4 + 1) * 512],
                                ps[:, c4 * 512 : (c4 + 1) * 512],
                            )

                    tailraw = pa.tile([128, T], bf16, name="tailraw", tag="tailraw")
                    tailsh = pa.tile([128, T], bf16, name="tailsh", tag="tailsh")
                    tm1 = pa.tile([128, T], bf16, name="tm1", tag="tm1")

                    def finish_main(ps, dst_list, tail_part, h):
                        # tail_part: 0 for q (tailraw rows 0:64), 1 for k
                        raw = pa.tile(
                            [128, T], bf16, name="raw", tag="raw", bufs=2
                        )
                        evac(ps, raw)
                        sh = pa.tile(
                            [128, T], bf16, name="sh", tag="sh", bufs=2
                        )
                        tb = 64 * tail_part + 16 * h
                        nc.gpsimd.dma_start(sh[0:56, :], raw[72:128, :])
                        nc.gpsimd.dma_start(sh[56:72, :], tailraw[tb : tb + 16, :])
                        nc.gpsimd.dma_start(sh[72:128, :], raw[0:56, :])
                        # stash rows 56:72 (partner of the tail dims)
                        nc.gpsimd.dma_start(
                            tailsh[tb : tb + 16, :], raw[56:72, :]
                        )
                        m1 = pa.tile([128, T], bf16, name="m1", tag="m1", bufs=2)
                        m2 = pa.tile([128, T], bf16, name="m2", tag="m2", bufs=2)
                        nc.vector.tensor_mul(m1[:], raw[:], cosm_sb[:])
                        nc.vector.tensor_mul(m2[:], sh[:], sinm_sb[:])
                        nc.vector.tensor_add(dst_list[h][:], m1[:], m2[:])

                    def tail_half(tail_part):
                        # rope this half of the tail block (k half unblocks
                        # the kTBz scatter long before the q mains finish);
                        # slices keep all operands at the same base partition
                        lo = 64 * tail_part
                        tm2 = pa.tile([128, T], bf16, name="tm2", tag="m2", bufs=2)
                        nc.vector.tensor_mul(
                            tm2[lo : lo + 64, :],
                            tailsh[lo : lo + 64, :],
                            sint_sb[lo : lo + 64, :],
                        )
                        nc.vector.tensor_add(
                            qkTB[lo : lo + 64, :],
                            tm1[lo : lo + 64, :],
                            tm2[lo : lo + 64, :],
                        )
                        if tail_part == 1:
                            for hh in range(HL):
                                nc.gpsimd.dma_start(
                                    kTBz[hh][16 * hh : 16 * hh + 16, :],
                                    qkTB[64 + 16 * hh : 64 + 16 * hh + 16, :],
                                )

                    # interleaved prologue: the tail block and k-main h=0
                    # stream x together, so the startup DMA bandwidth (x not
                    # yet resident) feeds two blocks' worth of matmuls
                    ps_t = paqps.tile([128, T], f32, name="psQ", tag="qkps", bufs=2)
                    ps_k0 = paqps.tile([128, T], f32, name="psQ", tag="qkps", bufs=2)
                    for k in range(KC):
                        st, sp = k == 0, k == KC - 1
                        for c4 in range(4):
                            sl = slice(c4 * 512, (c4 + 1) * 512)
                            nc.tensor.matmul(
                                ps_t[:, sl], wqk_t[k][:], xtiles[k][:, sl],
                                start=st, stop=sp,
                            )
                            nc.tensor.matmul(
                                ps_k0[:, sl], wm_t[k][:, 0:128], xtiles[k][:, sl],
                                start=st, stop=sp,
                            )
                    evac(ps_t, tailraw)
                    # tail cos-product is ready as soon as tailraw lands
                    nc.vector.tensor_mul(tm1[:], tailraw[:], cost_sb[:])
                    finish_main(ps_k0, kTa, 1, 0)
                    for h in range(1, HL):
                        ps = block_mm(
                            lambda k, h=h: wm_t[k][:, 128 * h : 128 * (h + 1)]
                        )
                        finish_main(ps, kTa, 1, h)
                    tail_half(1)
                    load_wm(wqM)
                    for h in range(HL):
                        ps = block_mm(
                            lambda k, h=h: wm_t[k][:, 128 * h : 128 * (h + 1)]
                        )
                        finish_main(ps, qTa, 0, h)
                    tail_half(0)

                # ---- V projection (x-chunk stationary, wv streaming) ----
                with (
                    tc.tile_pool(name="pav", bufs=1) as pav,
                    tc.tile_pool(name="pavps", bufs=1, space="PSUM") as pavps,
                ):
                    wv_t = []
                    for k in range(KC):
                        wt_ = pav.tile([128, DV], bf16, name=f"wv{k}", tag=f"wv{k}")
                        nc.sync.dma_start(wt_[:], wvT[k * 128 : (k + 1) * 128, :])
                        wv_t.append(wt_)
                    for t in range(NT):
                        psV = pavps.tile(
                            [128, DV], f32, name="psV", tag="vps", bufs=2
                        )
                        for k in range(KC):
                            st, sp = k == 0, k == KC - 1
                            lhs = xtiles[k][:, t * 128 : (t + 1) * 128]
                            nc.tensor.matmul(
                                psV[:, 0:512], lhs, wv_t[k][:, 0:512],
                                start=st, stop=sp,
                            )
                            nc.tensor.matmul(
                                psV[:, 512:DV], lhs, wv_t[k][:, 512:DV],
                                start=st, stop=sp,
                            )
                        v3 = vt[t].rearrange("p (h e) -> p h e", h=HL)
                        nc.any.tensor_copy(
                            v3[:, :, 0:HD],
                            psV.rearrange("p (h e) -> p h e", h=HL),
                        )
                        nc.vector.memset(v3[:, :, HD : HD + 1], 1.0)

            # ---------------- Phase B: attention --------------------------
            with tc.tile_pool(name="pb", bufs=1) as pb:
                ot = [
                    pb.tile([128, DV], bf16, name=f"o{t}", tag=f"o{t}")
                    for t in range(NT)
                ]
                with tc.tile_pool(name="pbps", bufs=1, space="PSUM") as pbps:
                    for qb in range(4):
                        for h in range(HL):
                            # pack the 4 q-tile accumulators into 2 PSUM banks:
                            # 3*145 fp32 = 1740B fits one 2KB bank
                            o_ps3 = pbps.tile(
                                [128, 3 * (HD + 1)], f32, name="o_ps3", tag="o3", bufs=1
                            )
                            o_ps1 = pbps.tile(
                                [128, HD + 1], f32, name="o_ps1", tag="o1", bufs=1
                            )
                            o_ps = [
                                o_ps3[:, 0 : HD + 1],
                                o_ps3[:, HD + 1 : 2 * (HD + 1)],
                                o_ps3[:, 2 * (HD + 1) : 3 * (HD + 1)],
                                o_ps1[:],
                            ]

                            def s_exp_group(g):
                                # 4 key-tiles per group, paired into 2-bank
                                # PSUM tiles; all matmuls K=128 (tail via
                                # zero-padded kTBz) -> no mode switches.
                                sps2 = [
                                    pbps.tile(
                                        [128, 1024], f32, name="sps", tag="sc", bufs=3
                                    )
                                    for _ in range(gsz // 2)
                                ]
                                for j in range(gsz):
                                    kt = gsz * g + j
                                    dst = sps2[j // 2][
                                        :, (j % 2) * 512 : (j % 2) * 512 + 512
                                    ]
                                    nc.tensor.matmul(
                                        dst,
                                        kTa[h][:, kt * 128 : (kt + 1) * 128],
                                        qTa[h][:, qb * 512 : (qb + 1) * 512],
                                        start=True,
                                        stop=False,
                                    )
                                for j in range(gsz):
                                    kt = gsz * g + j
                                    dst = sps2[j // 2][
                                        :, (j % 2) * 512 : (j % 2) * 512 + 512
                                    ]
                                    nc.tensor.matmul(
                                        dst,
                                        kTBz[h][:, kt * 128 : (kt + 1) * 128],
                                        qkTB[:, qb * 512 : (qb + 1) * 512],
                                        start=False,
                                        stop=True,
                                    )
                                Es = []
                                for j2 in range(gsz // 2):
                                    E = pb.tile(
                                        [128, 1024], bf16, name="E", tag="E", bufs=4
                                    )
                                    nc.scalar.activation(
                                        E[:], sps2[j2][:], AF.Exp, scale=SCALE
                                    )
                                    Es.append(E[:, 0:512])
                                    Es.append(E[:, 512:1024])
                                return Es

                            def pv_group(g, Es):
                                for j in range(gsz):
                                    kt = gsz * g + j
                                    for qt in range(4):
                                        # start/stop are bank-granular: qt 0-2
                                        # share o_ps3's bank
                                        if qt < 3:
                                            st = kt == 0 and qt == 0
                                            sp = kt == NT - 1 and qt == 2
                                        else:
                                            st = kt == 0
                                            sp = kt == NT - 1
                                        nc.tensor.matmul(
                                            o_ps[qt][:],
                                            Es[j][:, qt * 128 : (qt + 1) * 128],
                                            vt[kt][:, (HD + 1) * h : (HD + 1) * (h + 1)],
                                            start=st,
                                            stop=sp,
                                        )

                            ngrp = NT // gsz
                            Eprev = s_exp_group(0)
                            for g in range(ngrp):
                                Enext = s_exp_group(g + 1) if g + 1 < ngrp else None
                                pv_group(g, Eprev)
                                Eprev = Enext
                            for qt in range(4):
                                t = qb * 4 + qt
                                r = pb.tile([128, 1], f32, name="r", tag="r", bufs=4)
                                nc.vector.reciprocal(r[:], o_ps[qt][:, HD : HD + 1])
                                nc.vector.tensor_scalar_mul(
                                    ot[t][:, HD * h : HD * (h + 1)],
                                    o_ps[qt][:, 0:HD],
                                    r[:],
                                )

                # ---------------- Phase C: o^T + final projection ----------
                oTa = [
                    pb.tile([128, T], bf16, name=f"oTa{j}", tag=f"oTa{j}")
                    for j in range(4)
                ]
                oTb = pb.tile([128, T], bf16, name="oTb", tag="oTb")
                nc.vector.memset(oTb[64:128, :], 0.0)
                wo_tiles = []
                for k in range(5):
                    rows = 128 if k < 4 else 64
                    wot_ = pb.tile([128, D], bf16, name=f"wo{k}", tag=f"wo{k}")
                    nc.sync.dma_start(
                        wot_[0:rows, :], woT[k * 128 : k * 128 + rows, :]
                    )
                    if rows < 128:
                        nc.vector.memset(wot_[rows:128, :], 0.0)
                    wo_tiles.append(wot_)
                with tc.tile_pool(name="pcps", bufs=1, space="PSUM") as pcps:

                    def o_transp(t):
                        for j in range(4):
                            tp = pcps.tile(
                                [128, 128], bf16, name="tpo", tag="otp", bufs=2
                            )
                            nc.tensor.transpose(
                                tp[:],
                                ot[t][:, 128 * j : 128 * (j + 1)],
                                ident_bf[:],
                            )
                            nc.any.tensor_copy(
                                oTa[j][:, t * 128 : (t + 1) * 128], tp[:]
                            )
                        tpb = pcps.tile([64, 128], bf16, name="tpb", tag="otp", bufs=2)
                        nc.tensor.transpose(
                            tpb[:],
                            ot[t][:, 512:DV],
                            ident_bf[:],
                        )
                        nc.any.tensor_copy(
                            oTb[0:64, t * 128 : (t + 1) * 128], tpb[:]
                        )

                    def final(t):
                        # K-outer: each oT stationary's LDWEIGHTS hides
                        # under the previous chunk's 3 matmuls
                        fps3 = [
                            pcps.tile(
                                [128, 384], f32, name=f"fps{j3}", tag=f"f{j3}", bufs=2
                            )
                            for j3 in range(3)
                        ]
                        for k in range(5):
                            lhs = (
                                oTa[k][:, t * 128 : (t + 1) * 128]
                                if k < 4
                                else oTb[:, t * 128 : (t + 1) * 128]
                            )
                            for j3 in range(3):
                                nc.tensor.matmul(
                                    fps3[j3][:],
                                    lhs,
                                    wo_tiles[k][:, 384 * j3 : 384 * (j3 + 1)],
                                    start=(k == 0),
                                    stop=(k == 4),
                                )
                        for j3 in range(3):
                            fout = pb.tile(
                                [128, 384], f32, name="fout", tag="fout", bufs=4
                            )
                            nc.any.tensor_copy(fout[:], fps3[j3][:])
                            nc.sync.dma_start(
                                out[
                                    t * 128 : (t + 1) * 128,
                                    384 * j3 : 384 * (j3 + 1),
                                ],
                                fout[:],
                            )

                    o_transp(0)
                    for t in range(NT):
                        if t + 1 < NT:
                            o_transp(t + 1)
                        final(t)

    nc.compile()
    return nc


def get_nc(debug=False, gsz=None):
    key = (bool(debug), GSZ if gsz is None else gsz)
    if key not in _NC_CACHE:
        _NC_CACHE[key] = _build(debug, gsz)
    return _NC_CACHE[key]


def make_in_maps(x, cos, sin, Wq, Wk, Wv, Wo):
    import ml_dtypes

    bf = ml_dtypes.bfloat16
    x = np.asarray(x, np.float32)
    cos = np.asarray(cos, np.float32)
    sin = np.asarray(sin, np.float32)
    Wq, Wk, Wv, Wo = (np.asarray(w, np.float32) for w in (Wq, Wk, Wv, Wo))

    cosT = cos.T  # [144, T]
    sinT = sin.T
    sign = np.where(np.arange(128) < 72, -1.0, 1.0).astype(np.float32)
    cosmT = np.ascontiguousarray(cosT[0:128]).astype(bf)
    sinmT = np.ascontiguousarray(sinT[0:128] * sign[:, None]).astype(bf)
    tidx = 128 + (np.arange(128) % 16)
    costF = np.ascontiguousarray(cosT[tidx]).astype(bf)
    sintF = np.ascontiguousarray(sinT[tidx]).astype(bf)

    in_maps = []
    for c in range(NCORES):
        b, hg = divmod(c, 2)
        heads = [HL * hg + i for i in range(HL)]

        def main_w(W):
            sel = np.concatenate(
                [W[144 * g : 144 * g + 128] for g in heads], 0
            )  # [512, D]
            return np.ascontiguousarray(sel.T).astype(bf)

        qk_tail = np.zeros((128, D), np.float32)
        for i, g in enumerate(heads):
            qk_tail[16 * i : 16 * i + 16] = Wq[144 * g + 128 : 144 * g + 144]
            qk_tail[64 + 16 * i : 64 + 16 * i + 16] = Wk[144 * g + 128 : 144 * g + 144]

        wv_sel = np.concatenate([Wv[144 * g : 144 * g + 144] for g in heads], 0)
        wo_sel = np.concatenate([Wo[:, 144 * g : 144 * g + 144] for g in heads], 1)
        in_maps.append(
            {
                "xT": np.ascontiguousarray(x[b].T).astype(bf),
                "wqM": main_w(Wq),
                "wkM": main_w(Wk),
                "wqkT": np.ascontiguousarray(qk_tail.T).astype(bf),
                "wvT": np.ascontiguousarray(wv_sel.T).astype(bf),
                "woT": np.ascontiguousarray(wo_sel.T).astype(bf),
                "cosmT": cosmT,
                "sinmT": sinmT,
                "costF": costF,
                "sintF": sintF,
                "identB": np.eye(128, dtype=bf),
            }
        )
    return in_maps


def kernel(x, cos, sin, Wq, Wk, Wv, Wo, _trace=False, _trace_kwargs=None):
    from concourse.bass_utils import run_bass_kernel_spmd

    nc = get_nc()
    in_maps = make_in_maps(x, cos, sin, Wq, Wk, Wv, Wo)
    res = run_bass_kernel_spmd(
        nc,
        in_maps,
        list(range(NCORES)),
        trace=_trace,
        **(_trace_kwargs or {}),
    )
    parts = [res.results[c]["out"] for c in range(NCORES)]
    outb = np.stack([parts[2 * b] + parts[2 * b + 1] for b in range(B)])
    if _trace:
        kernel.last_results = res
    return outb.astype(np.float32)
